# revision 15
# baseline (speedup 1.0000x reference)
"""FDGNN (gnn_message_passing) Trainium2 kernel, 8-core SPMD.

Strategy:
- Only 3 of the reference's 6 convs feed the output:
    s1 = conv_i2s(xi0); i2 = conv_s2i(s1); s3 = conv_i2s(i2); out = tanh(s3@wo+bo)
- mlp_m commutes with the per-edge gather: mlp_m(x[src]) == mlp_m(x)[src], so
  the message MLP runs per *node* (12.5K rows/core), not per *edge*.
- Destination nodes are sharded across the 8 cores. Each conv:
    1. Y_local = mlp_m(x_local)  (PE, feature-major in, row-major out)
    2. AllGather Y -> message table [100352, 64] in DRAM
    3. dma_gather edge source rows (4 int16-indexed table chunks)
    4. segment-sum: psum_aggT[64, w128] += gathered_tile.T @ S_tile, where
       S_tile is a 0/1 dst-selection matrix built on DVE via iota compare.
       Edges are pre-routed on host into per-(window, chunk) cells padded to
       128-slot tiles (pad slots gather zero rows).
    5. x_local' = mlp_u(aggT)  (feature-major all the way)
- Final: out = tanh(x @ wo + bo) per 128-row tile via the lhsT flip trick.
"""

import numpy as np

NCORES = 8
NNODE = 100000  # both NS and NI
PERCORE = NNODE // NCORES  # 12500
NW = 98  # windows per core (98*128 = 12544)
PADPER = NW * 128  # 12544 padded rows per core
TABLE_ROWS = NCORES * PADPER  # 100352
NCHUNK = 4
CHUNK_ROWS = TABLE_ROWS // NCHUNK  # 25088 (< 32768, int16-safe)
D = 64
HM = 32
HU = 16
GT = 6  # tiles per dma_gather call
NQUEUES = 1  # SWDGE queues (1-4)
SB = 4  # tiles per S-build batch

TRACE = False  # set by test harness to capture an NTFF profile
LAST_RESULT = None  # BassKernelResults of the most recent run

import os

ABLATE = os.environ.get("KABL", "")  # "", "mlponly", "ag", "gather"


# ---------------------------------------------------------------- host prep

def _prep_relation(src, dst):
    """Route edges (dst-sharded) into per-core, per-chunk gather streams."""
    E = src.shape[0]
    src = src.astype(np.int64)
    dst = dst.astype(np.int64)

    p = dst // PERCORE
    dl = dst - p * PERCORE
    trow = (src // PERCORE) * PADPER + (src % PERCORE)
    c = trow // CHUNK_ROWS
    lidx = trow - c * CHUNK_ROWS
    w = dl >> 7
    drel = dl - (w << 7)

    key = (p * NCHUNK + c) * NW + w
    counts = np.bincount(key, minlength=NCORES * NCHUNK * NW).reshape(
        NCORES, NCHUNK, NW
    )
    ntiles_cw = -(-counts.max(axis=0) // 128)  # [NCHUNK, NW]
    ntiles_cw[0] = np.maximum(ntiles_cw[0], 1)  # every window has >=1 tile
    N_cw = ntiles_cw * 128
    base_w = np.zeros((NCHUNK, NW + 1), np.int64)
    base_w[:, 1:] = np.cumsum(N_cw, axis=1)
    T_c = (base_w[:, -1] // 128).astype(np.int64)  # tiles per chunk stream

    # rank of each edge within its (p, c, w) cell
    order = np.argsort(key, kind="stable")
    kk = key[order]
    grp_first = np.r_[True, kk[1:] != kk[:-1]]
    first_pos = np.flatnonzero(grp_first)
    starts = np.repeat(first_pos, np.diff(np.r_[first_pos, E]))
    rank = np.arange(E) - starts
    inv = np.empty(E, np.int64)
    inv[order] = rank
    slot = base_w[c, w] + inv  # slot within (core, chunk) stream

    idx_streams = []  # [core][chunk] -> int16 [128, T_c*8] packed
    drel_streams = []  # [core][chunk] -> fp32 [128, T_c]
    for pp in range(NCORES):
        rows_i = []
        rows_d = []
        pm = p == pp
        for cc in range(NCHUNK):
            n = int(T_c[cc]) * 128
            ar = np.arange(n)
            idx_flat = (12500 + (ar % 44) + (ar % 2) * PADPER).astype(np.int64)
            idx_flat = np.minimum(idx_flat, CHUNK_ROWS - 1)
            drel_flat = np.full(n, -1.0, np.float32)
            m = pm & (c == cc)
            idx_flat[slot[m]] = lidx[m]
            drel_flat[slot[m]] = drel[m]
            assert idx_flat.max() < CHUNK_ROWS and idx_flat.min() >= 0
            idx16 = idx_flat.astype(np.int16)
            packed = np.tile(idx16.reshape(n // 16, 16).T, (8, 1))  # [128, n/16]
            rows_i.append(np.ascontiguousarray(packed))
            rows_d.append(
                np.ascontiguousarray(drel_flat.reshape(-1, 128).T.astype(np.float32))
            )
        idx_streams.append(rows_i)
        drel_streams.append(rows_d)

    return {
        "ntiles_cw": ntiles_cw,  # [NCHUNK, NW]
        "T_c": T_c,  # [NCHUNK]
        "idx": idx_streams,
        "drel": drel_streams,
    }


# ---------------------------------------------------------------- program

def _build_program(meta_a, meta_b):
    """meta_a: i2s relation (convs 1 and 3), meta_b: s2i relation (conv 2)."""
    import concourse.mybir as mybir
    import concourse.tile as tile
    from concourse import bacc
    from concourse.bass import ts

    FP32 = mybir.dt.float32
    I16 = mybir.dt.int16
    AF = mybir.ActivationFunctionType

    nc = bacc.Bacc(
        "TRN2",
        target_bir_lowering=False,
        debug=False,
        enable_asserts=False,
        num_devices=NCORES,
        num_swdge_queues=NQUEUES,
    )

    # ---- I/O
    xi0T = nc.dram_tensor("xi0T", [D, PADPER], FP32, kind="ExternalInput")
    wm1 = nc.dram_tensor("wm1", [D, HM], FP32, kind="ExternalInput")
    bm1 = nc.dram_tensor("bm1", [HM, 1], FP32, kind="ExternalInput")
    wm2b = nc.dram_tensor("wm2b", [HM + 1, D], FP32, kind="ExternalInput")
    wu1 = nc.dram_tensor("wu1", [D, HU], FP32, kind="ExternalInput")
    bu1 = nc.dram_tensor("bu1", [HU, 1], FP32, kind="ExternalInput")
    wu2 = nc.dram_tensor("wu2", [HU, D], FP32, kind="ExternalInput")
    bu2 = nc.dram_tensor("bu2", [D, 1], FP32, kind="ExternalInput")
    wob = nc.dram_tensor("wob", [D + 1, D], FP32, kind="ExternalInput")

    idx_in = {}
    drel_in = {}
    for rel, meta in (("a", meta_a), ("b", meta_b)):
        for cc in range(NCHUNK):
            tcn = int(meta["T_c"][cc])
            idx_in[rel, cc] = nc.dram_tensor(
                f"idx_{rel}{cc}", [128, tcn * 8], I16, kind="ExternalInput"
            )
            drel_in[rel, cc] = nc.dram_tensor(
                f"drel_{rel}{cc}", [128, tcn], FP32, kind="ExternalInput"
            )

    out = nc.dram_tensor("out", [PADPER, D], FP32, kind="ExternalOutput")

    # collective buffers
    y_bounce = nc.dram_tensor("y_bounce", [PADPER, D], FP32)
    tables = [
        nc.dram_tensor(f"table{i}", [TABLE_ROWS, D], FP32, addr_space="Shared")
        for i in range(2)
    ]

    iota_np = np.tile(np.arange(128, dtype=np.float32), (128, 1))
    iota_dram = nc.inline_tensor(iota_np, name="iota")

    # PADPER = 12544 = 24*512 + 256
    col_tiles = [(i * 512, 512) for i in range(PADPER // 512)]
    if PADPER % 512:
        col_tiles.append((PADPER - PADPER % 512, PADPER % 512))

    with tile.TileContext(nc) as tc:
        with (
            tc.tile_pool(name="consts", bufs=1) as cs,
            tc.tile_pool(name="state", bufs=1) as st,
            tc.tile_pool(name="stage", bufs=3) as sg,
            tc.tile_pool(name="meta", bufs=2) as mp,
            tc.tile_pool(name="g0", bufs=2) as gp0,
            tc.tile_pool(name="g1", bufs=2) as gp1,
            tc.tile_pool(name="g2", bufs=2) as gp2,
            tc.tile_pool(name="g3", bufs=2) as gp3,
            tc.tile_pool(name="spool", bufs=2) as sp,
            tc.tile_pool(name="pw", bufs=2, space="PSUM") as pw,
            tc.tile_pool(name="pa", bufs=2, space="PSUM") as pa,
            tc.tile_pool(name="pb", bufs=2, space="PSUM") as pb,
            tc.tile_pool(name="pu", bufs=2, space="PSUM") as pu,
        ):
            gpools = [gp0, gp1, gp2, gp3]

            # ---- constants
            iota_s = cs.tile([128, 128], FP32)
            nc.sync.dma_start(out=iota_s[:], in_=iota_dram[:, :])
            wm1_s = cs.tile([D, HM], FP32)
            nc.sync.dma_start(out=wm1_s[:], in_=wm1[:, :])
            bm1_s = cs.tile([HM, 1], FP32)
            nc.sync.dma_start(out=bm1_s[:], in_=bm1[:, :])
            wm2b_s = cs.tile([HM + 1, D], FP32)
            nc.sync.dma_start(out=wm2b_s[:], in_=wm2b[:, :])
            wu1_s = cs.tile([D, HU], FP32)
            nc.sync.dma_start(out=wu1_s[:], in_=wu1[:, :])
            bu1_s = cs.tile([HU, 1], FP32)
            nc.sync.dma_start(out=bu1_s[:], in_=bu1[:, :])
            wu2_s = cs.tile([HU, D], FP32)
            nc.sync.dma_start(out=wu2_s[:], in_=wu2[:, :])
            bu2_s = cs.tile([D, 1], FP32)
            nc.sync.dma_start(out=bu2_s[:], in_=bu2[:, :])
            wob_s = cs.tile([D + 1, D], FP32)
            nc.sync.dma_start(out=wob_s[:], in_=wob[:, :])

            # ---- persistent state
            xT = st.tile([D + 1, PADPER], FP32)  # row D = ones
            nc.sync.dma_start(out=xT[0:D, :], in_=xi0T[:, :])
            nc.gpsimd.memset(xT[D : D + 1, :], 1.0)
            aggT = st.tile([D, PADPER], FP32)
            if ABLATE:
                nc.gpsimd.memset(aggT[:], 0.0)

            # zero rows of the y bounce buffer (pad rows 12500..12543), once
            zrow = cs.tile([44, D], FP32)
            nc.gpsimd.memset(zrow[:], 0.0)
            nc.sync.dma_start(out=y_bounce[12500:PADPER, :], in_=zrow[:])

            def mlp_m():
                """y_bounce = mlp_m(x) row-major via flipped second matmul."""
                h1_tiles = {}
                for c0, cn in col_tiles:
                    ps = pa.tile([HM, 512], FP32, tag="pa")
                    nc.tensor.matmul(
                        ps[:, :cn],
                        wm1_s[:],
                        xT[0:D, c0 : c0 + cn],
                        start=True,
                        stop=True,
                    )
                    h1 = sg.tile([HM + 1, 512], FP32, tag="h1")
                    nc.scalar.activation(
                        h1[0:HM, :cn], ps[:, :cn], AF.Relu, bias=bm1_s[:]
                    )
                    nc.gpsimd.memset(h1[HM : HM + 1, :cn], 1.0)
                    h1_tiles[c0] = h1
                    for j0 in range(0, cn, 128):
                        j = (c0 + j0) // 128
                        ps2 = pb.tile([128, D], FP32, tag="pb")
                        nc.tensor.matmul(
                            ps2[:],
                            h1[:, j0 : j0 + 128],
                            wm2b_s[:],
                            start=True,
                            stop=True,
                        )
                        ystage = sg.tile([128, D], FP32, tag="ystage")
                        nc.scalar.activation(ystage[:], ps2[:], AF.Relu)
                        r0 = j * 128
                        nrows = 128 if j < NW - 1 else (12500 - r0)
                        nc.sync.dma_start(
                            out=y_bounce[r0 : r0 + nrows, :], in_=ystage[0:nrows, :]
                        )

            def conv(meta, rel, table):
                """AllGather y -> table; gather + segment-sum -> aggT."""
                if ABLATE == "mlponly":
                    return
                nc.gpsimd.collective_compute(
                    "AllGather",
                    mybir.AluOpType.bypass,
                    replica_groups=[list(range(NCORES))],
                    ins=[y_bounce.ap().opt()],
                    outs=[table.ap().opt()],
                )

                if ABLATE == "ag":
                    return
                ntiles_cw = meta["ntiles_cw"]
                T_c = meta["T_c"]

                # drel streams loaded whole (small)
                drel_s = []
                for cc in range(NCHUNK):
                    tcn = int(T_c[cc])
                    dt_ = mp.tile([128, tcn], FP32, tag=f"drel{cc}")
                    nc.sync.dma_start(out=dt_[:], in_=drel_in[rel, cc][:, :])
                    drel_s.append(dt_)

                calls = []
                for cc in range(NCHUNK):
                    tcn = int(T_c[cc])
                    calls.append(
                        [(t0, min(GT, tcn - t0)) for t0 in range(0, tcn, GT)]
                    )

                gbufs = [None] * NCHUNK
                gcall = [-1] * NCHUNK
                sbufs = [None] * NCHUNK
                sbatch = [-1] * NCHUNK

                def ensure_gather(cc, t):
                    k = t // GT
                    if gcall[cc] != k:
                        t0, nt = calls[cc][k]
                        ix = mp.tile([128, nt * 8], I16, tag=f"idx{cc}")
                        nc.sync.dma_start(
                            out=ix[:],
                            in_=idx_in[rel, cc][:, t0 * 8 : (t0 + nt) * 8],
                        )
                        gb = gpools[cc].tile([128, nt, D], FP32, tag=f"gb{cc}")
                        nc.gpsimd.dma_gather(
                            gb[:],
                            table[cc * CHUNK_ROWS : (cc + 1) * CHUNK_ROWS, :],
                            ix[:],
                            nt * 128,
                            nt * 128,
                            D,
                            elem_step=D,
                            queue_num=cc % NQUEUES,
                        )
                        gbufs[cc] = gb
                        gcall[cc] = k
                    return gbufs[cc], t - calls[cc][k][0]

                def ensure_s(cc, t):
                    k = t // SB
                    if sbatch[cc] != k:
                        t0 = k * SB
                        nb = min(SB, int(T_c[cc]) - t0)
                        stile = sp.tile([128, SB, 128], FP32, tag=f"sb{cc}")
                        nc.vector.tensor_tensor(
                            out=stile[:, 0:nb, :],
                            in0=drel_s[cc][:, t0 : t0 + nb].to_broadcast(
                                [128, nb, 128]
                            ),
                            in1=iota_s[:]
                            .rearrange("p (o w) -> p o w", o=1)
                            .to_broadcast([128, nb, 128]),
                            op=mybir.AluOpType.is_equal,
                        )
                        sbufs[cc] = stile
                        sbatch[cc] = k
                    return sbufs[cc], t - k * SB

                if ABLATE == "gather":
                    for cc in range(NCHUNK):
                        for t in range(0, int(T_c[cc]), GT):
                            ensure_gather(cc, t)
                    return
                tile_cursor = [0] * NCHUNK
                for w in range(NW):
                    total_mms = int(ntiles_cw[:, w].sum())
                    ps = pw.tile([D, 128], FP32, tag="pw")
                    mm = 0
                    for cc in range(NCHUNK):
                        for _ in range(int(ntiles_cw[cc, w])):
                            t = tile_cursor[cc]
                            gb, gslot = ensure_gather(cc, t)
                            stile, sslot = ensure_s(cc, t)
                            nc.tensor.matmul(
                                ps[:],
                                gb[:, gslot, :],
                                stile[:, sslot, :],
                                start=(mm == 0),
                                stop=(mm == total_mms - 1),
                            )
                            tile_cursor[cc] += 1
                            mm += 1
                    nc.vector.tensor_copy(out=aggT[:, ts(w, 128)], in_=ps[:])

            def mlp_u():
                """xT = relu(wu2.T @ relu(wu1.T @ aggT + bu1) + bu2)."""
                for c0, cn in col_tiles:
                    ps1 = pu.tile([D, 512], FP32, tag="pu")
                    nc.tensor.matmul(
                        ps1[0:HU, :cn],
                        wu1_s[:],
                        aggT[:, c0 : c0 + cn],
                        start=True,
                        stop=True,
                    )
                    hu = sg.tile([HU, 512], FP32, tag="hu")
                    nc.scalar.activation(
                        hu[:, :cn], ps1[0:HU, :cn], AF.Relu, bias=bu1_s[:]
                    )
                    ps2 = pu.tile([D, 512], FP32, tag="pu")
                    nc.tensor.matmul(
                        ps2[:, :cn], wu2_s[:], hu[:, :cn], start=True, stop=True
                    )
                    nc.scalar.activation(
                        xT[0:D, c0 : c0 + cn], ps2[:, :cn], AF.Relu, bias=bu2_s[:]
                    )

            # ---------------- the 3 convs
            mlp_m()
            conv(meta_a, "a", tables[0])
            mlp_u()

            mlp_m()
            conv(meta_b, "b", tables[1])
            mlp_u()

            mlp_m()
            conv(meta_a, "a", tables[0])
            mlp_u()

            # ---------------- final h2o
            for j in range(NW):
                ps = pb.tile([128, D], FP32, tag="pb")
                nc.tensor.matmul(
                    ps[:], xT[:, ts(j, 128)], wob_s[:], start=True, stop=True
                )
                ostage = sg.tile([128, D], FP32, tag="ostage")
                nc.scalar.activation(ostage[:], ps[:], AF.Tanh)
                nc.sync.dma_start(out=out[ts(j, 128), :], in_=ostage[:])

    nc.compile()
    return nc


# ---------------------------------------------------------------- entry

def _prepare(
    x_served,
    x_interfered,
    edge_s2i,
    edge_i2s,
    wm1,
    bm1,
    wm2,
    bm2,
    wu1,
    bu1,
    wu2,
    bu2,
    wo,
    bo,
):
    """Host prep + program build. Returns (nc, in_maps)."""
    x_interfered = np.asarray(x_interfered, np.float32)
    e_s2i = np.asarray(edge_s2i)
    e_i2s = np.asarray(edge_i2s)

    # relation a: i2s (src interfered, dst served) -- convs 1 and 3
    meta_a = _prep_relation(e_i2s[0], e_i2s[1])
    # relation b: s2i (src served, dst interfered) -- conv 2
    meta_b = _prep_relation(e_s2i[0], e_s2i[1])

    nc = _build_program(meta_a, meta_b)

    wm2b = np.concatenate([wm2, bm2[None, :]], axis=0).astype(np.float32)
    wob = np.concatenate([wo, bo[None, :]], axis=0).astype(np.float32)

    in_maps = []
    for p in range(NCORES):
        xi_loc = np.zeros((D, PADPER), np.float32)
        xi_loc[:, :PERCORE] = x_interfered[p * PERCORE : (p + 1) * PERCORE].T
        m = {
            "xi0T": xi_loc,
            "wm1": np.ascontiguousarray(np.asarray(wm1, np.float32)),
            "bm1": np.ascontiguousarray(np.asarray(bm1, np.float32).reshape(HM, 1)),
            "wm2b": wm2b,
            "wu1": np.ascontiguousarray(np.asarray(wu1, np.float32)),
            "bu1": np.ascontiguousarray(np.asarray(bu1, np.float32).reshape(HU, 1)),
            "wu2": np.ascontiguousarray(np.asarray(wu2, np.float32)),
            "bu2": np.ascontiguousarray(np.asarray(bu2, np.float32).reshape(D, 1)),
            "wob": wob,
        }
        for rel, meta in (("a", meta_a), ("b", meta_b)):
            for cc in range(NCHUNK):
                m[f"idx_{rel}{cc}"] = meta["idx"][p][cc]
                m[f"drel_{rel}{cc}"] = meta["drel"][p][cc]
        in_maps.append(m)

    return nc, in_maps


def kernel(**inputs):
    from concourse.bass_utils import run_bass_kernel_spmd

    nc, in_maps = _prepare(**inputs)
    res = run_bass_kernel_spmd(
        nc, in_maps, core_ids=list(range(NCORES)), trace=TRACE
    )
    global LAST_RESULT
    LAST_RESULT = res
    outs = [res.results[p]["out"][:PERCORE] for p in range(NCORES)]
    return np.concatenate(outs, axis=0)


# revision 16
# speedup vs baseline: 1.1023x; 1.1023x over previous
"""FDGNN (gnn_message_passing) Trainium2 kernel, 8-core SPMD.

Strategy:
- Only 3 of the reference's 6 convs feed the output:
    s1 = conv_i2s(xi0); i2 = conv_s2i(s1); s3 = conv_i2s(i2); out = tanh(s3@wo+bo)
- mlp_m commutes with the per-edge gather: mlp_m(x[src]) == mlp_m(x)[src], so
  the message MLP runs per *node* (12.5K rows/core), not per *edge*.
- Destination nodes are sharded across the 8 cores. Each conv:
    1. Y_local = mlp_m(x_local)  (PE, feature-major in, row-major out)
    2. AllGather Y -> message table [100352, 64] in DRAM
    3. dma_gather edge source rows (4 int16-indexed table chunks)
    4. segment-sum: psum_aggT[64, w128] += gathered_tile.T @ S_tile, where
       S_tile is a 0/1 dst-selection matrix built on DVE via iota compare.
       Edges are pre-routed on host into per-(window, chunk) cells padded to
       128-slot tiles (pad slots gather zero rows).
    5. x_local' = mlp_u(aggT)  (feature-major all the way)
- Final: out = tanh(x @ wo + bo) per 128-row tile via the lhsT flip trick.
"""

import numpy as np

NCORES = 8
NNODE = 100000  # both NS and NI
PERCORE = NNODE // NCORES  # 12500
NW = 98  # windows per core (98*128 = 12544)
PADPER = NW * 128  # 12544 padded rows per core
TABLE_ROWS = NCORES * PADPER  # 100352
NCHUNK = 4
CHUNK_ROWS = TABLE_ROWS // NCHUNK  # 25088 (< 32768, int16-safe)
D = 64
HM = 32
HU = 16
GT = 6  # tiles per dma_gather call
NQUEUES = 1  # SWDGE queues (1-4)
SB = 4  # tiles per S-build batch

TRACE = False  # set by test harness to capture an NTFF profile
LAST_RESULT = None  # BassKernelResults of the most recent run

import os

ABLATE = os.environ.get("KABL", "")  # "", "mlponly", "ag", "gather"


# ---------------------------------------------------------------- host prep

def _prep_relation(src, dst):
    """Route edges (dst-sharded) into per-core, per-chunk gather streams."""
    E = src.shape[0]
    src = src.astype(np.int64)
    dst = dst.astype(np.int64)

    p = dst // PERCORE
    dl = dst - p * PERCORE
    trow = (src // PERCORE) * PADPER + (src % PERCORE)
    c = trow // CHUNK_ROWS
    lidx = trow - c * CHUNK_ROWS
    w = dl >> 7
    drel = dl - (w << 7)

    key = (p * NCHUNK + c) * NW + w
    counts = np.bincount(key, minlength=NCORES * NCHUNK * NW).reshape(
        NCORES, NCHUNK, NW
    )
    ntiles_cw = -(-counts.max(axis=0) // 128)  # [NCHUNK, NW]
    ntiles_cw[0] = np.maximum(ntiles_cw[0], 1)  # every window has >=1 tile
    N_cw = ntiles_cw * 128
    base_w = np.zeros((NCHUNK, NW + 1), np.int64)
    base_w[:, 1:] = np.cumsum(N_cw, axis=1)
    T_c = (base_w[:, -1] // 128).astype(np.int64)  # tiles per chunk stream

    # rank of each edge within its (p, c, w) cell
    order = np.argsort(key, kind="stable")
    kk = key[order]
    grp_first = np.r_[True, kk[1:] != kk[:-1]]
    first_pos = np.flatnonzero(grp_first)
    starts = np.repeat(first_pos, np.diff(np.r_[first_pos, E]))
    rank = np.arange(E) - starts
    inv = np.empty(E, np.int64)
    inv[order] = rank
    slot = base_w[c, w] + inv  # slot within (core, chunk) stream

    idx_streams = []  # [core][chunk] -> int16 [128, T_c*8] packed
    drel_streams = []  # [core][chunk] -> fp32 [128, T_c]
    for pp in range(NCORES):
        rows_i = []
        rows_d = []
        pm = p == pp
        for cc in range(NCHUNK):
            n = int(T_c[cc]) * 128
            ar = np.arange(n)
            idx_flat = (12500 + (ar % 44) + (ar % 2) * PADPER).astype(np.int64)
            idx_flat = np.minimum(idx_flat, CHUNK_ROWS - 1)
            drel_flat = np.full(n, -1.0, np.float32)
            m = pm & (c == cc)
            idx_flat[slot[m]] = lidx[m]
            drel_flat[slot[m]] = drel[m]
            assert idx_flat.max() < CHUNK_ROWS and idx_flat.min() >= 0
            idx16 = idx_flat.astype(np.int16)
            packed = np.tile(idx16.reshape(n // 16, 16).T, (8, 1))  # [128, n/16]
            rows_i.append(np.ascontiguousarray(packed))
            rows_d.append(
                np.ascontiguousarray(drel_flat.reshape(-1, 128).T.astype(np.float32))
            )
        idx_streams.append(rows_i)
        drel_streams.append(rows_d)

    return {
        "ntiles_cw": ntiles_cw,  # [NCHUNK, NW]
        "T_c": T_c,  # [NCHUNK]
        "idx": idx_streams,
        "drel": drel_streams,
    }


# ---------------------------------------------------------------- program

def _build_program(meta_a, meta_b):
    """meta_a: i2s relation (convs 1 and 3), meta_b: s2i relation (conv 2)."""
    import concourse.mybir as mybir
    import concourse.tile as tile
    from concourse import bacc
    from concourse.bass import ts

    FP32 = mybir.dt.float32
    BF16 = mybir.dt.bfloat16
    I16 = mybir.dt.int16
    AF = mybir.ActivationFunctionType

    nc = bacc.Bacc(
        "TRN2",
        target_bir_lowering=False,
        debug=False,
        enable_asserts=False,
        num_devices=NCORES,
        num_swdge_queues=NQUEUES,
    )

    # ---- I/O
    xi0T = nc.dram_tensor("xi0T", [D, PADPER], FP32, kind="ExternalInput")
    wm1 = nc.dram_tensor("wm1", [D, HM], FP32, kind="ExternalInput")
    bm1 = nc.dram_tensor("bm1", [HM, 1], FP32, kind="ExternalInput")
    wm2b = nc.dram_tensor("wm2b", [HM + 1, D], FP32, kind="ExternalInput")
    wu1 = nc.dram_tensor("wu1", [D, HU], FP32, kind="ExternalInput")
    bu1 = nc.dram_tensor("bu1", [HU, 1], FP32, kind="ExternalInput")
    wu2 = nc.dram_tensor("wu2", [HU, D], FP32, kind="ExternalInput")
    bu2 = nc.dram_tensor("bu2", [D, 1], FP32, kind="ExternalInput")
    wob = nc.dram_tensor("wob", [D + 1, D], FP32, kind="ExternalInput")

    idx_in = {}
    drel_in = {}
    for rel, meta in (("a", meta_a), ("b", meta_b)):
        for cc in range(NCHUNK):
            tcn = int(meta["T_c"][cc])
            idx_in[rel, cc] = nc.dram_tensor(
                f"idx_{rel}{cc}", [128, tcn * 8], I16, kind="ExternalInput"
            )
            drel_in[rel, cc] = nc.dram_tensor(
                f"drel_{rel}{cc}", [128, tcn], FP32, kind="ExternalInput"
            )

    out = nc.dram_tensor("out", [PADPER, D], FP32, kind="ExternalOutput")

    # collective buffers; rows hold the 64 bf16 features twice (256B granule)
    y_bounce = nc.dram_tensor("y_bounce", [PADPER, 2 * D], BF16)
    tables = [
        nc.dram_tensor(f"table{i}", [TABLE_ROWS, 2 * D], BF16, addr_space="Shared")
        for i in range(2)
    ]

    iota_np = np.tile(np.arange(128, dtype=np.float32), (128, 1))
    iota_dram = nc.inline_tensor(iota_np, name="iota")

    # PADPER = 12544 = 24*512 + 256
    col_tiles = [(i * 512, 512) for i in range(PADPER // 512)]
    if PADPER % 512:
        col_tiles.append((PADPER - PADPER % 512, PADPER % 512))

    with tile.TileContext(nc) as tc:
        with (
            tc.tile_pool(name="consts", bufs=1) as cs,
            tc.tile_pool(name="state", bufs=1) as st,
            tc.tile_pool(name="stage", bufs=3) as sg,
            tc.tile_pool(name="meta", bufs=2) as mp,
            tc.tile_pool(name="g0", bufs=2) as gp0,
            tc.tile_pool(name="g1", bufs=2) as gp1,
            tc.tile_pool(name="g2", bufs=2) as gp2,
            tc.tile_pool(name="g3", bufs=2) as gp3,
            tc.tile_pool(name="spool", bufs=2) as sp,
            tc.tile_pool(name="pw", bufs=2, space="PSUM") as pw,
            tc.tile_pool(name="pa", bufs=2, space="PSUM") as pa,
            tc.tile_pool(name="pb", bufs=2, space="PSUM") as pb,
            tc.tile_pool(name="pu", bufs=2, space="PSUM") as pu,
        ):
            gpools = [gp0, gp1, gp2, gp3]

            # ---- constants
            iota_s = cs.tile([128, 128], FP32)
            nc.sync.dma_start(out=iota_s[:], in_=iota_dram[:, :])
            wm1_s = cs.tile([D, HM], FP32)
            nc.sync.dma_start(out=wm1_s[:], in_=wm1[:, :])
            bm1_s = cs.tile([HM, 1], FP32)
            nc.sync.dma_start(out=bm1_s[:], in_=bm1[:, :])
            wm2b_s = cs.tile([HM + 1, D], FP32)
            nc.sync.dma_start(out=wm2b_s[:], in_=wm2b[:, :])
            wu1_s = cs.tile([D, HU], FP32)
            nc.sync.dma_start(out=wu1_s[:], in_=wu1[:, :])
            bu1_s = cs.tile([HU, 1], FP32)
            nc.sync.dma_start(out=bu1_s[:], in_=bu1[:, :])
            wu2_s = cs.tile([HU, D], FP32)
            nc.sync.dma_start(out=wu2_s[:], in_=wu2[:, :])
            bu2_s = cs.tile([D, 1], FP32)
            nc.sync.dma_start(out=bu2_s[:], in_=bu2[:, :])
            wob_s = cs.tile([D + 1, D], FP32)
            nc.sync.dma_start(out=wob_s[:], in_=wob[:, :])

            # ---- persistent state
            xT = st.tile([D + 1, PADPER], FP32)  # row D = ones
            nc.sync.dma_start(out=xT[0:D, :], in_=xi0T[:, :])
            nc.gpsimd.memset(xT[D : D + 1, :], 1.0)
            aggT = st.tile([D, PADPER], FP32)
            if ABLATE:
                nc.gpsimd.memset(aggT[:], 0.0)

            # zero rows of the y bounce buffer (pad rows 12500..12543), once
            zrow = cs.tile([44, 2 * D], BF16)
            nc.gpsimd.memset(zrow[:], 0.0)
            nc.sync.dma_start(out=y_bounce[12500:PADPER, :], in_=zrow[:])

            def mlp_m():
                """y_bounce = mlp_m(x) row-major via flipped second matmul."""
                h1_tiles = {}
                for c0, cn in col_tiles:
                    ps = pa.tile([HM, 512], FP32, tag="pa")
                    nc.tensor.matmul(
                        ps[:, :cn],
                        wm1_s[:],
                        xT[0:D, c0 : c0 + cn],
                        start=True,
                        stop=True,
                    )
                    h1 = sg.tile([HM + 1, 512], FP32, tag="h1")
                    nc.scalar.activation(
                        h1[0:HM, :cn], ps[:, :cn], AF.Relu, bias=bm1_s[:]
                    )
                    nc.gpsimd.memset(h1[HM : HM + 1, :cn], 1.0)
                    h1_tiles[c0] = h1
                    for j0 in range(0, cn, 128):
                        j = (c0 + j0) // 128
                        ps2 = pb.tile([128, D], FP32, tag="pb")
                        nc.tensor.matmul(
                            ps2[:],
                            h1[:, j0 : j0 + 128],
                            wm2b_s[:],
                            start=True,
                            stop=True,
                        )
                        ystage = sg.tile([128, D], BF16, tag="ystage")
                        nc.scalar.activation(ystage[:], ps2[:], AF.Relu)
                        r0 = j * 128
                        nrows = 128 if j < NW - 1 else (12500 - r0)
                        nc.sync.dma_start(
                            out=y_bounce[r0 : r0 + nrows, 0:D], in_=ystage[0:nrows, :]
                        )
                        nc.sync.dma_start(
                            out=y_bounce[r0 : r0 + nrows, D : 2 * D],
                            in_=ystage[0:nrows, :],
                        )

            def conv(meta, rel, table):
                """AllGather y -> table; gather + segment-sum -> aggT."""
                if ABLATE == "mlponly":
                    return
                nc.gpsimd.collective_compute(
                    "AllGather",
                    mybir.AluOpType.bypass,
                    replica_groups=[list(range(NCORES))],
                    ins=[y_bounce.ap().opt()],
                    outs=[table.ap().opt()],
                )

                if ABLATE == "ag":
                    return
                ntiles_cw = meta["ntiles_cw"]
                T_c = meta["T_c"]

                # drel streams loaded whole (small)
                drel_s = []
                for cc in range(NCHUNK):
                    tcn = int(T_c[cc])
                    dt_ = mp.tile([128, tcn], FP32, tag=f"drel{cc}")
                    nc.sync.dma_start(out=dt_[:], in_=drel_in[rel, cc][:, :])
                    drel_s.append(dt_)

                calls = []
                for cc in range(NCHUNK):
                    tcn = int(T_c[cc])
                    calls.append(
                        [(t0, min(GT, tcn - t0)) for t0 in range(0, tcn, GT)]
                    )

                gbufs = [None] * NCHUNK
                gcall = [-1] * NCHUNK
                sbufs = [None] * NCHUNK
                sbatch = [-1] * NCHUNK

                def ensure_gather(cc, t):
                    k = t // GT
                    if gcall[cc] != k:
                        t0, nt = calls[cc][k]
                        ix = mp.tile([128, nt * 8], I16, tag=f"idx{cc}")
                        nc.sync.dma_start(
                            out=ix[:],
                            in_=idx_in[rel, cc][:, t0 * 8 : (t0 + nt) * 8],
                        )
                        gb = gpools[cc].tile([128, nt, 2 * D], BF16, tag=f"gb{cc}")
                        nc.gpsimd.dma_gather(
                            gb[:],
                            table[cc * CHUNK_ROWS : (cc + 1) * CHUNK_ROWS, :],
                            ix[:],
                            nt * 128,
                            nt * 128,
                            2 * D,
                            elem_step=2 * D,
                            queue_num=cc % NQUEUES,
                        )
                        gbufs[cc] = gb
                        gcall[cc] = k
                    return gbufs[cc], t - calls[cc][k][0]

                def ensure_s(cc, t):
                    k = t // SB
                    if sbatch[cc] != k:
                        t0 = k * SB
                        nb = min(SB, int(T_c[cc]) - t0)
                        stile = sp.tile([128, SB, 128], BF16, tag=f"sb{cc}")
                        nc.vector.tensor_tensor(
                            out=stile[:, 0:nb, :],
                            in0=drel_s[cc][:, t0 : t0 + nb].to_broadcast(
                                [128, nb, 128]
                            ),
                            in1=iota_s[:]
                            .rearrange("p (o w) -> p o w", o=1)
                            .to_broadcast([128, nb, 128]),
                            op=mybir.AluOpType.is_equal,
                        )
                        sbufs[cc] = stile
                        sbatch[cc] = k
                    return sbufs[cc], t - k * SB

                if ABLATE == "gather":
                    for cc in range(NCHUNK):
                        for t in range(0, int(T_c[cc]), GT):
                            ensure_gather(cc, t)
                    return
                tile_cursor = [0] * NCHUNK
                for w in range(NW):
                    total_mms = int(ntiles_cw[:, w].sum())
                    ps = pw.tile([128, 128], FP32, tag="pw")
                    mm = 0
                    for cc in range(NCHUNK):
                        for _ in range(int(ntiles_cw[cc, w])):
                            t = tile_cursor[cc]
                            gb, gslot = ensure_gather(cc, t)
                            stile, sslot = ensure_s(cc, t)
                            nc.tensor.matmul(
                                ps[:],
                                gb[:, gslot, :],
                                stile[:, sslot, :],
                                start=(mm == 0),
                                stop=(mm == total_mms - 1),
                            )
                            tile_cursor[cc] += 1
                            mm += 1
                    nc.vector.tensor_copy(out=aggT[:, ts(w, 128)], in_=ps[0:D, :])

            def mlp_u():
                """xT = relu(wu2.T @ relu(wu1.T @ aggT + bu1) + bu2)."""
                for c0, cn in col_tiles:
                    ps1 = pu.tile([D, 512], FP32, tag="pu")
                    nc.tensor.matmul(
                        ps1[0:HU, :cn],
                        wu1_s[:],
                        aggT[:, c0 : c0 + cn],
                        start=True,
                        stop=True,
                    )
                    hu = sg.tile([HU, 512], FP32, tag="hu")
                    nc.scalar.activation(
                        hu[:, :cn], ps1[0:HU, :cn], AF.Relu, bias=bu1_s[:]
                    )
                    ps2 = pu.tile([D, 512], FP32, tag="pu")
                    nc.tensor.matmul(
                        ps2[:, :cn], wu2_s[:], hu[:, :cn], start=True, stop=True
                    )
                    nc.scalar.activation(
                        xT[0:D, c0 : c0 + cn], ps2[:, :cn], AF.Relu, bias=bu2_s[:]
                    )

            # ---------------- the 3 convs
            mlp_m()
            conv(meta_a, "a", tables[0])
            mlp_u()

            mlp_m()
            conv(meta_b, "b", tables[1])
            mlp_u()

            mlp_m()
            conv(meta_a, "a", tables[0])
            mlp_u()

            # ---------------- final h2o
            for j in range(NW):
                ps = pb.tile([128, D], FP32, tag="pb")
                nc.tensor.matmul(
                    ps[:], xT[:, ts(j, 128)], wob_s[:], start=True, stop=True
                )
                ostage = sg.tile([128, D], FP32, tag="ostage")
                nc.scalar.activation(ostage[:], ps[:], AF.Tanh)
                nc.sync.dma_start(out=out[ts(j, 128), :], in_=ostage[:])

    nc.compile()
    return nc


# ---------------------------------------------------------------- entry

def _prepare(
    x_served,
    x_interfered,
    edge_s2i,
    edge_i2s,
    wm1,
    bm1,
    wm2,
    bm2,
    wu1,
    bu1,
    wu2,
    bu2,
    wo,
    bo,
):
    """Host prep + program build. Returns (nc, in_maps)."""
    x_interfered = np.asarray(x_interfered, np.float32)
    e_s2i = np.asarray(edge_s2i)
    e_i2s = np.asarray(edge_i2s)

    # relation a: i2s (src interfered, dst served) -- convs 1 and 3
    meta_a = _prep_relation(e_i2s[0], e_i2s[1])
    # relation b: s2i (src served, dst interfered) -- conv 2
    meta_b = _prep_relation(e_s2i[0], e_s2i[1])

    nc = _build_program(meta_a, meta_b)

    wm2b = np.concatenate([wm2, bm2[None, :]], axis=0).astype(np.float32)
    wob = np.concatenate([wo, bo[None, :]], axis=0).astype(np.float32)

    in_maps = []
    for p in range(NCORES):
        xi_loc = np.zeros((D, PADPER), np.float32)
        xi_loc[:, :PERCORE] = x_interfered[p * PERCORE : (p + 1) * PERCORE].T
        m = {
            "xi0T": xi_loc,
            "wm1": np.ascontiguousarray(np.asarray(wm1, np.float32)),
            "bm1": np.ascontiguousarray(np.asarray(bm1, np.float32).reshape(HM, 1)),
            "wm2b": wm2b,
            "wu1": np.ascontiguousarray(np.asarray(wu1, np.float32)),
            "bu1": np.ascontiguousarray(np.asarray(bu1, np.float32).reshape(HU, 1)),
            "wu2": np.ascontiguousarray(np.asarray(wu2, np.float32)),
            "bu2": np.ascontiguousarray(np.asarray(bu2, np.float32).reshape(D, 1)),
            "wob": wob,
        }
        for rel, meta in (("a", meta_a), ("b", meta_b)):
            for cc in range(NCHUNK):
                m[f"idx_{rel}{cc}"] = meta["idx"][p][cc]
                m[f"drel_{rel}{cc}"] = meta["drel"][p][cc]
        in_maps.append(m)

    return nc, in_maps


def kernel(**inputs):
    from concourse.bass_utils import run_bass_kernel_spmd

    nc, in_maps = _prepare(**inputs)
    res = run_bass_kernel_spmd(
        nc, in_maps, core_ids=list(range(NCORES)), trace=TRACE
    )
    global LAST_RESULT
    LAST_RESULT = res
    outs = [res.results[p]["out"][:PERCORE] for p in range(NCORES)]
    return np.concatenate(outs, axis=0)


# revision 18
# speedup vs baseline: 1.1649x; 1.0568x over previous
"""FDGNN (gnn_message_passing) Trainium2 kernel, 8-core SPMD.

Strategy:
- Only 3 of the reference's 6 convs feed the output:
    s1 = conv_i2s(xi0); i2 = conv_s2i(s1); s3 = conv_i2s(i2); out = tanh(s3@wo+bo)
- mlp_m commutes with the per-edge gather: mlp_m(x[src]) == mlp_m(x)[src], so
  the message MLP runs per *node* (12.5K rows/core), not per *edge*.
- Destination nodes are sharded across the 8 cores. Each conv:
    1. Y_local = mlp_m(x_local)  (PE, feature-major in, row-major out)
    2. AllGather Y -> message table [100352, 64] in DRAM
    3. dma_gather edge source rows (4 int16-indexed table chunks)
    4. segment-sum: psum_aggT[64, w128] += gathered_tile.T @ S_tile, where
       S_tile is a 0/1 dst-selection matrix built on DVE via iota compare.
       Edges are pre-routed on host into per-(window, chunk) cells padded to
       128-slot tiles (pad slots gather zero rows).
    5. x_local' = mlp_u(aggT)  (feature-major all the way)
- Final: out = tanh(x @ wo + bo) per 128-row tile via the lhsT flip trick.
"""

import numpy as np

NCORES = 8
NNODE = 100000  # both NS and NI
PERCORE = NNODE // NCORES  # 12500
NW = 98  # windows per core (98*128 = 12544)
PADPER = NW * 128  # 12544 padded rows per core
TABLE_ROWS = NCORES * PADPER  # 100352
NCHUNK = 4
CHUNK_ROWS = TABLE_ROWS // NCHUNK  # 25088 (< 32768, int16-safe)
D = 64
HM = 32
HU = 16
import os as _os

GT = int(_os.environ.get("KGT", "6"))  # tiles per dma_gather call
NQUEUES = int(_os.environ.get("KNQ", "1"))  # SWDGE queues (1-4)
SCRATCH = int(_os.environ.get("KSCRATCH", "16384"))
SB = 4  # tiles per S-build batch

TRACE = False  # set by test harness to capture an NTFF profile
LAST_RESULT = None  # BassKernelResults of the most recent run

import os

ABLATE = os.environ.get("KABL", "")  # "", "mlponly", "ag", "gather"


# ---------------------------------------------------------------- host prep

def _prep_relation(src, dst):
    """Route edges (dst-sharded) into per-core, per-chunk gather streams."""
    E = src.shape[0]
    src = src.astype(np.int64)
    dst = dst.astype(np.int64)

    p = dst // PERCORE
    dl = dst - p * PERCORE
    trow = (src // PERCORE) * PADPER + (src % PERCORE)
    c = trow // CHUNK_ROWS
    lidx = trow - c * CHUNK_ROWS
    w = dl >> 7
    drel = dl - (w << 7)

    key = (p * NCHUNK + c) * NW + w
    counts = np.bincount(key, minlength=NCORES * NCHUNK * NW).reshape(
        NCORES, NCHUNK, NW
    )
    ntiles_cw = -(-counts.max(axis=0) // 128)  # [NCHUNK, NW]
    ntiles_cw[0] = np.maximum(ntiles_cw[0], 1)  # every window has >=1 tile
    N_cw = ntiles_cw * 128
    base_w = np.zeros((NCHUNK, NW + 1), np.int64)
    base_w[:, 1:] = np.cumsum(N_cw, axis=1)
    T_c = (base_w[:, -1] // 128).astype(np.int64)  # tiles per chunk stream

    # rank of each edge within its (p, c, w) cell
    order = np.argsort(key, kind="stable")
    kk = key[order]
    grp_first = np.r_[True, kk[1:] != kk[:-1]]
    first_pos = np.flatnonzero(grp_first)
    starts = np.repeat(first_pos, np.diff(np.r_[first_pos, E]))
    rank = np.arange(E) - starts
    inv = np.empty(E, np.int64)
    inv[order] = rank
    slot = base_w[c, w] + inv  # slot within (core, chunk) stream

    idx_streams = []  # [core][chunk] -> int16 [128, T_c*8] packed
    drel_streams = []  # [core][chunk] -> fp32 [128, T_c]
    for pp in range(NCORES):
        rows_i = []
        rows_d = []
        pm = p == pp
        for cc in range(NCHUNK):
            n = int(T_c[cc]) * 128
            ar = np.arange(n)
            idx_flat = (12500 + (ar % 44) + (ar % 2) * PADPER).astype(np.int64)
            idx_flat = np.minimum(idx_flat, CHUNK_ROWS - 1)
            drel_flat = np.full(n, -1.0, np.float32)
            m = pm & (c == cc)
            idx_flat[slot[m]] = lidx[m]
            drel_flat[slot[m]] = drel[m]
            assert idx_flat.max() < CHUNK_ROWS and idx_flat.min() >= 0
            idx16 = idx_flat.astype(np.int16)
            packed = np.tile(idx16.reshape(n // 16, 16).T, (8, 1))  # [128, n/16]
            rows_i.append(np.ascontiguousarray(packed))
            rows_d.append(
                np.ascontiguousarray(drel_flat.reshape(-1, 128).T.astype(np.float32))
            )
        idx_streams.append(rows_i)
        drel_streams.append(rows_d)

    return {
        "ntiles_cw": ntiles_cw,  # [NCHUNK, NW]
        "T_c": T_c,  # [NCHUNK]
        "idx": idx_streams,
        "drel": drel_streams,
    }


# ---------------------------------------------------------------- program

def _build_program(meta_a, meta_b):
    """meta_a: i2s relation (convs 1 and 3), meta_b: s2i relation (conv 2)."""
    import concourse.mybir as mybir
    import concourse.tile as tile
    from concourse import bacc
    from concourse.bass import ts

    FP32 = mybir.dt.float32
    BF16 = mybir.dt.bfloat16
    I16 = mybir.dt.int16
    AF = mybir.ActivationFunctionType

    nc = bacc.Bacc(
        "TRN2",
        target_bir_lowering=False,
        debug=False,
        enable_asserts=False,
        num_devices=NCORES,
        num_swdge_queues=NQUEUES,
        dynamic_dma_scratch_size=SCRATCH,
    )

    # ---- I/O
    xi0T = nc.dram_tensor("xi0T", [D, PADPER], FP32, kind="ExternalInput")
    wm1 = nc.dram_tensor("wm1", [D, HM], FP32, kind="ExternalInput")
    bm1 = nc.dram_tensor("bm1", [HM, 1], FP32, kind="ExternalInput")
    wm2b = nc.dram_tensor("wm2b", [HM + 1, D], FP32, kind="ExternalInput")
    wu1 = nc.dram_tensor("wu1", [D, HU], FP32, kind="ExternalInput")
    bu1 = nc.dram_tensor("bu1", [HU, 1], FP32, kind="ExternalInput")
    wu2 = nc.dram_tensor("wu2", [HU, D], FP32, kind="ExternalInput")
    bu2 = nc.dram_tensor("bu2", [D, 1], FP32, kind="ExternalInput")
    wob = nc.dram_tensor("wob", [D + 1, D], FP32, kind="ExternalInput")

    idx_in = {}
    drel_in = {}
    for rel, meta in (("a", meta_a), ("b", meta_b)):
        for cc in range(NCHUNK):
            tcn = int(meta["T_c"][cc])
            idx_in[rel, cc] = nc.dram_tensor(
                f"idx_{rel}{cc}", [128, tcn * 8], I16, kind="ExternalInput"
            )
            drel_in[rel, cc] = nc.dram_tensor(
                f"drel_{rel}{cc}", [128, tcn], FP32, kind="ExternalInput"
            )

    out = nc.dram_tensor("out", [PADPER, D], FP32, kind="ExternalOutput")

    # collective buffers; rows hold the 64 bf16 features twice (256B granule)
    y_bounce = nc.dram_tensor("y_bounce", [PADPER, 2 * D], BF16)
    tables = [
        nc.dram_tensor(f"table{i}", [TABLE_ROWS, 2 * D], BF16, addr_space="Shared")
        for i in range(2)
    ]

    iota_np = np.tile(np.arange(128, dtype=np.float32), (128, 1))
    iota_dram = nc.inline_tensor(iota_np, name="iota")

    # PADPER = 12544 = 24*512 + 256
    col_tiles = [(i * 512, 512) for i in range(PADPER // 512)]
    if PADPER % 512:
        col_tiles.append((PADPER - PADPER % 512, PADPER % 512))

    with tile.TileContext(nc) as tc:
        with (
            tc.tile_pool(name="consts", bufs=1) as cs,
            tc.tile_pool(name="state", bufs=1) as st,
            tc.tile_pool(name="stage", bufs=3) as sg,
            tc.tile_pool(name="meta", bufs=2) as mp,
            tc.tile_pool(name="g0", bufs=2) as gp0,
            tc.tile_pool(name="g1", bufs=2) as gp1,
            tc.tile_pool(name="g2", bufs=2) as gp2,
            tc.tile_pool(name="g3", bufs=2) as gp3,
            tc.tile_pool(name="spool", bufs=2) as sp,
            tc.tile_pool(name="pw", bufs=2, space="PSUM") as pw,
            tc.tile_pool(name="pa", bufs=2, space="PSUM") as pa,
            tc.tile_pool(name="pb", bufs=2, space="PSUM") as pb,
            tc.tile_pool(name="pu", bufs=2, space="PSUM") as pu,
        ):
            gpools = [gp0, gp1, gp2, gp3]

            # ---- constants
            iota_s = cs.tile([128, 128], FP32)
            nc.sync.dma_start(out=iota_s[:], in_=iota_dram[:, :])
            wm1_s = cs.tile([D, HM], FP32)
            nc.sync.dma_start(out=wm1_s[:], in_=wm1[:, :])
            bm1_s = cs.tile([HM, 1], FP32)
            nc.sync.dma_start(out=bm1_s[:], in_=bm1[:, :])
            wm2b_s = cs.tile([HM + 1, D], FP32)
            nc.sync.dma_start(out=wm2b_s[:], in_=wm2b[:, :])
            wu1_s = cs.tile([D, HU], FP32)
            nc.sync.dma_start(out=wu1_s[:], in_=wu1[:, :])
            bu1_s = cs.tile([HU, 1], FP32)
            nc.sync.dma_start(out=bu1_s[:], in_=bu1[:, :])
            wu2_s = cs.tile([HU, D], FP32)
            nc.sync.dma_start(out=wu2_s[:], in_=wu2[:, :])
            bu2_s = cs.tile([D, 1], FP32)
            nc.sync.dma_start(out=bu2_s[:], in_=bu2[:, :])
            wob_s = cs.tile([D + 1, D], FP32)
            nc.sync.dma_start(out=wob_s[:], in_=wob[:, :])

            # ---- persistent state
            xT = st.tile([D + 1, PADPER], FP32)  # row D = ones
            nc.sync.dma_start(out=xT[0:D, :], in_=xi0T[:, :])
            nc.gpsimd.memset(xT[D : D + 1, :], 1.0)
            aggT = st.tile([D, PADPER], FP32)
            if ABLATE:
                nc.gpsimd.memset(aggT[:], 0.0)

            # zero rows of the y bounce buffer (pad rows 12500..12543), once
            zrow = cs.tile([44, 2 * D], BF16)
            nc.gpsimd.memset(zrow[:], 0.0)
            nc.sync.dma_start(out=y_bounce[12500:PADPER, :], in_=zrow[:])

            def mlp_m():
                """y_bounce = mlp_m(x) row-major via flipped second matmul."""
                h1_tiles = {}
                for c0, cn in col_tiles:
                    ps = pa.tile([HM, 512], FP32, tag="pa")
                    nc.tensor.matmul(
                        ps[:, :cn],
                        wm1_s[:],
                        xT[0:D, c0 : c0 + cn],
                        start=True,
                        stop=True,
                    )
                    h1 = sg.tile([HM + 1, 512], FP32, tag="h1")
                    nc.scalar.activation(
                        h1[0:HM, :cn], ps[:, :cn], AF.Relu, bias=bm1_s[:]
                    )
                    nc.gpsimd.memset(h1[HM : HM + 1, :cn], 1.0)
                    h1_tiles[c0] = h1
                    for j0 in range(0, cn, 128):
                        j = (c0 + j0) // 128
                        ps2 = pb.tile([128, D], FP32, tag="pb")
                        nc.tensor.matmul(
                            ps2[:],
                            h1[:, j0 : j0 + 128],
                            wm2b_s[:],
                            start=True,
                            stop=True,
                        )
                        ystage = sg.tile([128, D], BF16, tag="ystage")
                        nc.scalar.activation(ystage[:], ps2[:], AF.Relu)
                        r0 = j * 128
                        nrows = 128 if j < NW - 1 else (12500 - r0)
                        nc.sync.dma_start(
                            out=y_bounce[r0 : r0 + nrows, 0:D], in_=ystage[0:nrows, :]
                        )
                        nc.sync.dma_start(
                            out=y_bounce[r0 : r0 + nrows, D : 2 * D],
                            in_=ystage[0:nrows, :],
                        )

            def conv(meta, rel, table):
                """AllGather y -> table; gather + segment-sum -> aggT."""
                if ABLATE == "mlponly":
                    return
                nc.gpsimd.collective_compute(
                    "AllGather",
                    mybir.AluOpType.bypass,
                    replica_groups=[list(range(NCORES))],
                    ins=[y_bounce.ap().opt()],
                    outs=[table.ap().opt()],
                )

                if ABLATE == "ag":
                    return
                ntiles_cw = meta["ntiles_cw"]
                T_c = meta["T_c"]

                # drel streams loaded whole (small)
                drel_s = []
                for cc in range(NCHUNK):
                    tcn = int(T_c[cc])
                    dt_ = mp.tile([128, tcn], FP32, tag=f"drel{cc}")
                    nc.sync.dma_start(out=dt_[:], in_=drel_in[rel, cc][:, :])
                    drel_s.append(dt_)

                calls = []
                for cc in range(NCHUNK):
                    tcn = int(T_c[cc])
                    calls.append(
                        [(t0, min(GT, tcn - t0)) for t0 in range(0, tcn, GT)]
                    )

                gbufs = [None] * NCHUNK
                gcall = [-1] * NCHUNK
                sbufs = [None] * NCHUNK
                sbatch = [-1] * NCHUNK

                def ensure_gather(cc, t):
                    k = t // GT
                    if gcall[cc] != k:
                        t0, nt = calls[cc][k]
                        ix = mp.tile([128, nt * 8], I16, tag=f"idx{cc}")
                        nc.sync.dma_start(
                            out=ix[:],
                            in_=idx_in[rel, cc][:, t0 * 8 : (t0 + nt) * 8],
                        )
                        gb = gpools[cc].tile([128, nt, 2 * D], BF16, tag=f"gb{cc}")
                        nc.gpsimd.dma_gather(
                            gb[:],
                            table[cc * CHUNK_ROWS : (cc + 1) * CHUNK_ROWS, :],
                            ix[:],
                            nt * 128,
                            nt * 128,
                            2 * D,
                            elem_step=2 * D,
                            queue_num=cc % NQUEUES,
                            single_packet=GT <= 8,
                        )
                        gbufs[cc] = gb
                        gcall[cc] = k
                    return gbufs[cc], t - calls[cc][k][0]

                def ensure_s(cc, t):
                    k = t // SB
                    if sbatch[cc] != k:
                        t0 = k * SB
                        nb = min(SB, int(T_c[cc]) - t0)
                        stile = sp.tile([128, SB, 128], BF16, tag=f"sb{cc}")
                        nc.vector.tensor_tensor(
                            out=stile[:, 0:nb, :],
                            in0=drel_s[cc][:, t0 : t0 + nb].to_broadcast(
                                [128, nb, 128]
                            ),
                            in1=iota_s[:]
                            .rearrange("p (o w) -> p o w", o=1)
                            .to_broadcast([128, nb, 128]),
                            op=mybir.AluOpType.is_equal,
                        )
                        sbufs[cc] = stile
                        sbatch[cc] = k
                    return sbufs[cc], t - k * SB

                if ABLATE == "gather":
                    for cc in range(NCHUNK):
                        for t in range(0, int(T_c[cc]), GT):
                            ensure_gather(cc, t)
                    return
                tile_cursor = [0] * NCHUNK
                for w in range(NW):
                    total_mms = int(ntiles_cw[:, w].sum())
                    ps = pw.tile([128, 128], FP32, tag="pw")
                    mm = 0
                    for cc in range(NCHUNK):
                        for _ in range(int(ntiles_cw[cc, w])):
                            t = tile_cursor[cc]
                            gb, gslot = ensure_gather(cc, t)
                            stile, sslot = ensure_s(cc, t)
                            nc.tensor.matmul(
                                ps[:],
                                gb[:, gslot, :],
                                stile[:, sslot, :],
                                start=(mm == 0),
                                stop=(mm == total_mms - 1),
                            )
                            tile_cursor[cc] += 1
                            mm += 1
                    nc.vector.tensor_copy(out=aggT[:, ts(w, 128)], in_=ps[0:D, :])

            def mlp_u():
                """xT = relu(wu2.T @ relu(wu1.T @ aggT + bu1) + bu2)."""
                for c0, cn in col_tiles:
                    ps1 = pu.tile([D, 512], FP32, tag="pu")
                    nc.tensor.matmul(
                        ps1[0:HU, :cn],
                        wu1_s[:],
                        aggT[:, c0 : c0 + cn],
                        start=True,
                        stop=True,
                    )
                    hu = sg.tile([HU, 512], FP32, tag="hu")
                    nc.scalar.activation(
                        hu[:, :cn], ps1[0:HU, :cn], AF.Relu, bias=bu1_s[:]
                    )
                    ps2 = pu.tile([D, 512], FP32, tag="pu")
                    nc.tensor.matmul(
                        ps2[:, :cn], wu2_s[:], hu[:, :cn], start=True, stop=True
                    )
                    nc.scalar.activation(
                        xT[0:D, c0 : c0 + cn], ps2[:, :cn], AF.Relu, bias=bu2_s[:]
                    )

            # ---------------- the 3 convs
            mlp_m()
            conv(meta_a, "a", tables[0])
            mlp_u()

            mlp_m()
            conv(meta_b, "b", tables[1])
            mlp_u()

            mlp_m()
            conv(meta_a, "a", tables[0])
            mlp_u()

            # ---------------- final h2o
            for j in range(NW):
                ps = pb.tile([128, D], FP32, tag="pb")
                nc.tensor.matmul(
                    ps[:], xT[:, ts(j, 128)], wob_s[:], start=True, stop=True
                )
                ostage = sg.tile([128, D], FP32, tag="ostage")
                nc.scalar.activation(ostage[:], ps[:], AF.Tanh)
                nc.sync.dma_start(out=out[ts(j, 128), :], in_=ostage[:])

    nc.compile()
    return nc


# ---------------------------------------------------------------- entry

def _prepare(
    x_served,
    x_interfered,
    edge_s2i,
    edge_i2s,
    wm1,
    bm1,
    wm2,
    bm2,
    wu1,
    bu1,
    wu2,
    bu2,
    wo,
    bo,
):
    """Host prep + program build. Returns (nc, in_maps)."""
    x_interfered = np.asarray(x_interfered, np.float32)
    e_s2i = np.asarray(edge_s2i)
    e_i2s = np.asarray(edge_i2s)

    # relation a: i2s (src interfered, dst served) -- convs 1 and 3
    meta_a = _prep_relation(e_i2s[0], e_i2s[1])
    # relation b: s2i (src served, dst interfered) -- conv 2
    meta_b = _prep_relation(e_s2i[0], e_s2i[1])

    nc = _build_program(meta_a, meta_b)

    wm2b = np.concatenate([wm2, bm2[None, :]], axis=0).astype(np.float32)
    wob = np.concatenate([wo, bo[None, :]], axis=0).astype(np.float32)

    in_maps = []
    for p in range(NCORES):
        xi_loc = np.zeros((D, PADPER), np.float32)
        xi_loc[:, :PERCORE] = x_interfered[p * PERCORE : (p + 1) * PERCORE].T
        m = {
            "xi0T": xi_loc,
            "wm1": np.ascontiguousarray(np.asarray(wm1, np.float32)),
            "bm1": np.ascontiguousarray(np.asarray(bm1, np.float32).reshape(HM, 1)),
            "wm2b": wm2b,
            "wu1": np.ascontiguousarray(np.asarray(wu1, np.float32)),
            "bu1": np.ascontiguousarray(np.asarray(bu1, np.float32).reshape(HU, 1)),
            "wu2": np.ascontiguousarray(np.asarray(wu2, np.float32)),
            "bu2": np.ascontiguousarray(np.asarray(bu2, np.float32).reshape(D, 1)),
            "wob": wob,
        }
        for rel, meta in (("a", meta_a), ("b", meta_b)):
            for cc in range(NCHUNK):
                m[f"idx_{rel}{cc}"] = meta["idx"][p][cc]
                m[f"drel_{rel}{cc}"] = meta["drel"][p][cc]
        in_maps.append(m)

    return nc, in_maps


def kernel(**inputs):
    from concourse.bass_utils import run_bass_kernel_spmd

    nc, in_maps = _prepare(**inputs)
    res = run_bass_kernel_spmd(
        nc, in_maps, core_ids=list(range(NCORES)), trace=TRACE
    )
    global LAST_RESULT
    LAST_RESULT = res
    outs = [res.results[p]["out"][:PERCORE] for p in range(NCORES)]
    return np.concatenate(outs, axis=0)


# revision 20
# speedup vs baseline: 2.0279x; 1.7407x over previous
"""FDGNN (gnn_message_passing) Trainium2 kernel, 8-core SPMD.

Strategy:
- Only 3 of the reference's 6 convs feed the output:
    s1 = conv_i2s(xi0); i2 = conv_s2i(s1); s3 = conv_i2s(i2); out = tanh(s3@wo+bo)
- mlp_m commutes with the per-edge gather: mlp_m(x[src]) == mlp_m(x)[src], so
  the message MLP runs per *node* (12.5K rows/core), not per *edge*.
- Destination nodes are sharded across the 8 cores. Each conv:
    1. Y_local = mlp_m(x_local)  (PE, feature-major in, row-major out)
    2. AllGather Y -> message table [100352, 64] in DRAM
    3. dma_gather edge source rows (4 int16-indexed table chunks)
    4. segment-sum: psum_aggT[64, w128] += gathered_tile.T @ S_tile, where
       S_tile is a 0/1 dst-selection matrix built on DVE via iota compare.
       Edges are pre-routed on host into per-(window, chunk) cells padded to
       128-slot tiles (pad slots gather zero rows).
    5. x_local' = mlp_u(aggT)  (feature-major all the way)
- Final: out = tanh(x @ wo + bo) per 128-row tile via the lhsT flip trick.
"""

import numpy as np

NCORES = 8
NNODE = 100000  # both NS and NI
PERCORE = NNODE // NCORES  # 12500
NW = 98  # windows per core (98*128 = 12544)
PADPER = NW * 128  # 12544 padded rows per core
TABLE_ROWS = NCORES * PADPER  # 100352
NCHUNK = 4
CHUNK_ROWS = TABLE_ROWS // NCHUNK  # 25088 (< 32768, int16-safe)
D = 64
HM = 32
HU = 16
import os as _os

GT = int(_os.environ.get("KGT", "6"))  # tiles per dma_gather call
NQUEUES = int(_os.environ.get("KNQ", "1"))  # SWDGE queues (1-4)
SCRATCH = int(_os.environ.get("KSCRATCH", "16384"))
SB = 4  # tiles per S-build batch

TRACE = False  # set by test harness to capture an NTFF profile
LAST_RESULT = None  # BassKernelResults of the most recent run

import os

ABLATE = os.environ.get("KABL", "")  # "", "mlponly", "ag", "gather"


# ---------------------------------------------------------------- host prep

def _prep_relation(src, dst):
    """Route edges (dst-sharded) into per-core, per-chunk gather streams."""
    E = src.shape[0]
    src = src.astype(np.int64)
    dst = dst.astype(np.int64)

    p = dst // PERCORE
    dl = dst - p * PERCORE
    trow = (src // PERCORE) * PADPER + (src % PERCORE)
    c = trow // CHUNK_ROWS
    lidx = trow - c * CHUNK_ROWS
    w = dl >> 7
    drel = dl - (w << 7)

    key = (p * NCHUNK + c) * NW + w
    counts = np.bincount(key, minlength=NCORES * NCHUNK * NW).reshape(
        NCORES, NCHUNK, NW
    )
    ntiles_cw = -(-counts.max(axis=0) // 128)  # [NCHUNK, NW]
    ntiles_cw[0] = np.maximum(ntiles_cw[0], 1)  # every window has >=1 tile
    N_cw = ntiles_cw * 128
    base_w = np.zeros((NCHUNK, NW + 1), np.int64)
    base_w[:, 1:] = np.cumsum(N_cw, axis=1)
    T_c = (base_w[:, -1] // 128).astype(np.int64)  # tiles per chunk stream

    # rank of each edge within its (p, c, w) cell
    order = np.argsort(key, kind="stable")
    kk = key[order]
    grp_first = np.r_[True, kk[1:] != kk[:-1]]
    first_pos = np.flatnonzero(grp_first)
    starts = np.repeat(first_pos, np.diff(np.r_[first_pos, E]))
    rank = np.arange(E) - starts
    inv = np.empty(E, np.int64)
    inv[order] = rank
    slot = base_w[c, w] + inv  # slot within (core, chunk) stream

    idx_streams = []  # [core][chunk] -> int16 [128, T_c*8] packed
    drel_streams = []  # [core][chunk] -> fp32 [128, T_c]
    for pp in range(NCORES):
        rows_i = []
        rows_d = []
        pm = p == pp
        for cc in range(NCHUNK):
            n = int(T_c[cc]) * 128
            ar = np.arange(n)
            idx_flat = (12500 + (ar % 44) + (ar % 2) * PADPER).astype(np.int64)
            idx_flat = np.minimum(idx_flat, CHUNK_ROWS - 1)
            drel_flat = np.full(n, -1.0, np.float32)
            m = pm & (c == cc)
            idx_flat[slot[m]] = lidx[m]
            drel_flat[slot[m]] = drel[m]
            assert idx_flat.max() < CHUNK_ROWS and idx_flat.min() >= 0
            idx16 = idx_flat.astype(np.int16)
            packed = np.tile(idx16.reshape(n // 16, 16).T, (8, 1))  # [128, n/16]
            rows_i.append(np.ascontiguousarray(packed))
            rows_d.append(
                np.ascontiguousarray(drel_flat.reshape(-1, 128).T.astype(np.float32))
            )
        idx_streams.append(rows_i)
        drel_streams.append(rows_d)

    return {
        "ntiles_cw": ntiles_cw,  # [NCHUNK, NW]
        "T_c": T_c,  # [NCHUNK]
        "idx": idx_streams,
        "drel": drel_streams,
    }


# ---------------------------------------------------------------- program

def _build_program(meta_a, meta_b):
    """meta_a: i2s relation (convs 1 and 3), meta_b: s2i relation (conv 2)."""
    import concourse.mybir as mybir
    import concourse.tile as tile
    from concourse import bacc
    from concourse.bass import ts

    FP32 = mybir.dt.float32
    BF16 = mybir.dt.bfloat16
    I16 = mybir.dt.int16
    AF = mybir.ActivationFunctionType

    nc = bacc.Bacc(
        "TRN2",
        target_bir_lowering=False,
        debug=False,
        enable_asserts=False,
        num_devices=NCORES,
        num_swdge_queues=NQUEUES,
        dynamic_dma_scratch_size=SCRATCH,
    )

    # ---- I/O
    xi0T = nc.dram_tensor("xi0T", [D, PADPER], FP32, kind="ExternalInput")
    wm1 = nc.dram_tensor("wm1", [D, HM], FP32, kind="ExternalInput")
    bm1 = nc.dram_tensor("bm1", [HM, 1], FP32, kind="ExternalInput")
    wm2b = nc.dram_tensor("wm2b", [HM + 1, D], FP32, kind="ExternalInput")
    wu1 = nc.dram_tensor("wu1", [D, HU], FP32, kind="ExternalInput")
    bu1 = nc.dram_tensor("bu1", [HU, 1], FP32, kind="ExternalInput")
    wu2 = nc.dram_tensor("wu2", [HU, D], FP32, kind="ExternalInput")
    bu2 = nc.dram_tensor("bu2", [D, 1], FP32, kind="ExternalInput")
    wob = nc.dram_tensor("wob", [D + 1, D], FP32, kind="ExternalInput")

    idx_in = {}
    drel_in = {}
    for rel, meta in (("a", meta_a), ("b", meta_b)):
        for cc in range(NCHUNK):
            tcn = int(meta["T_c"][cc])
            idx_in[rel, cc] = nc.dram_tensor(
                f"idx_{rel}{cc}", [128, tcn * 8], I16, kind="ExternalInput"
            )
            drel_in[rel, cc] = nc.dram_tensor(
                f"drel_{rel}{cc}", [128, tcn], FP32, kind="ExternalInput"
            )

    out = nc.dram_tensor("out", [PADPER, D], FP32, kind="ExternalOutput")

    # collective buffers; rows hold the 64 bf16 features twice (256B granule)
    y_bounce = nc.dram_tensor("y_bounce", [PADPER, 2 * D], BF16)
    tables = [
        nc.dram_tensor(f"table{i}", [TABLE_ROWS, 2 * D], BF16, addr_space="Shared")
        for i in range(2)
    ]

    iota_np = np.tile(np.arange(128, dtype=np.float32), (128, 1))
    iota_dram = nc.inline_tensor(iota_np, name="iota")
    import ml_dtypes

    ones_dram = nc.inline_tensor(np.ones((1, PADPER), np.float32), name="onesrow")
    zeros_dram = nc.inline_tensor(
        np.zeros((44, 2 * D), ml_dtypes.bfloat16), name="zerorows"
    )

    # PADPER = 12544 = 24*512 + 256
    col_tiles = [(i * 512, 512) for i in range(PADPER // 512)]
    if PADPER % 512:
        col_tiles.append((PADPER - PADPER % 512, PADPER % 512))

    with tile.TileContext(nc) as tc:
        with (
            tc.tile_pool(name="consts", bufs=1) as cs,
            tc.tile_pool(name="state", bufs=1) as st,
            tc.tile_pool(name="stage", bufs=3) as sg,
            tc.tile_pool(name="meta", bufs=2) as mp,
            tc.tile_pool(name="g0", bufs=2) as gp0,
            tc.tile_pool(name="g1", bufs=2) as gp1,
            tc.tile_pool(name="g2", bufs=2) as gp2,
            tc.tile_pool(name="g3", bufs=2) as gp3,
            tc.tile_pool(name="spool", bufs=2) as sp,
            tc.tile_pool(name="pw", bufs=2, space="PSUM") as pw,
            tc.tile_pool(name="pa", bufs=2, space="PSUM") as pa,
            tc.tile_pool(name="pb", bufs=2, space="PSUM") as pb,
            tc.tile_pool(name="pu", bufs=2, space="PSUM") as pu,
        ):
            gpools = [gp0, gp1, gp2, gp3]

            # ---- constants
            iota_s = cs.tile([128, 128], FP32)
            nc.sync.dma_start(out=iota_s[:], in_=iota_dram[:, :])
            wm1_s = cs.tile([D, HM], FP32)
            nc.sync.dma_start(out=wm1_s[:], in_=wm1[:, :])
            bm1_s = cs.tile([HM, 1], FP32)
            nc.sync.dma_start(out=bm1_s[:], in_=bm1[:, :])
            wm2b_s = cs.tile([HM + 1, D], FP32)
            nc.sync.dma_start(out=wm2b_s[:], in_=wm2b[:, :])
            wu1_s = cs.tile([D, HU], FP32)
            nc.sync.dma_start(out=wu1_s[:], in_=wu1[:, :])
            bu1_s = cs.tile([HU, 1], FP32)
            nc.sync.dma_start(out=bu1_s[:], in_=bu1[:, :])
            wu2_s = cs.tile([HU, D], FP32)
            nc.sync.dma_start(out=wu2_s[:], in_=wu2[:, :])
            bu2_s = cs.tile([D, 1], FP32)
            nc.sync.dma_start(out=bu2_s[:], in_=bu2[:, :])
            wob_s = cs.tile([D + 1, D], FP32)
            nc.sync.dma_start(out=wob_s[:], in_=wob[:, :])

            # ---- persistent state
            xT = st.tile([D + 1, PADPER], FP32)  # row D = ones
            nc.sync.dma_start(out=xT[0:D, :], in_=xi0T[:, :])
            nc.sync.dma_start(out=xT[D : D + 1, :], in_=ones_dram[:, :])
            aggT = st.tile([D, PADPER], FP32)
            if ABLATE:
                nc.gpsimd.memset(aggT[:], 0.0)

            # zero rows of the y bounce buffer (pad rows 12500..12543), once
            zrow = cs.tile([44, 2 * D], BF16)
            nc.sync.dma_start(out=zrow[:], in_=zeros_dram[:, :])
            nc.sync.dma_start(out=y_bounce[12500:PADPER, :], in_=zrow[:])

            def mlp_m():
                """y_bounce = mlp_m(x) row-major via flipped second matmul."""
                h1_tiles = {}
                for c0, cn in col_tiles:
                    ps = pa.tile([HM, 512], FP32, tag="pa")
                    nc.tensor.matmul(
                        ps[:, :cn],
                        wm1_s[:],
                        xT[0:D, c0 : c0 + cn],
                        start=True,
                        stop=True,
                    )
                    h1 = sg.tile([HM + 1, 512], FP32, tag="h1")
                    nc.scalar.activation(
                        h1[0:HM, :cn], ps[:, :cn], AF.Relu, bias=bm1_s[:]
                    )
                    nc.sync.dma_start(
                        out=h1[HM : HM + 1, :cn], in_=ones_dram[:, 0:cn]
                    )
                    h1_tiles[c0] = h1
                    for j0 in range(0, cn, 128):
                        j = (c0 + j0) // 128
                        ps2 = pb.tile([128, D], FP32, tag="pb")
                        nc.tensor.matmul(
                            ps2[:],
                            h1[:, j0 : j0 + 128],
                            wm2b_s[:],
                            start=True,
                            stop=True,
                        )
                        ystage = sg.tile([128, D], BF16, tag="ystage")
                        nc.scalar.activation(ystage[:], ps2[:], AF.Relu)
                        r0 = j * 128
                        nrows = 128 if j < NW - 1 else (12500 - r0)
                        nc.sync.dma_start(
                            out=y_bounce[r0 : r0 + nrows, 0:D], in_=ystage[0:nrows, :]
                        )
                        nc.sync.dma_start(
                            out=y_bounce[r0 : r0 + nrows, D : 2 * D],
                            in_=ystage[0:nrows, :],
                        )

            def conv(meta, rel, table):
                """AllGather y -> table; gather + segment-sum -> aggT."""
                if ABLATE == "mlponly":
                    return
                nc.gpsimd.collective_compute(
                    "AllGather",
                    mybir.AluOpType.bypass,
                    replica_groups=[list(range(NCORES))],
                    ins=[y_bounce.ap().opt()],
                    outs=[table.ap().opt()],
                )

                if ABLATE == "ag":
                    return
                ntiles_cw = meta["ntiles_cw"]
                T_c = meta["T_c"]

                # drel streams loaded whole (small)
                drel_s = []
                for cc in range(NCHUNK):
                    tcn = int(T_c[cc])
                    dt_ = mp.tile([128, tcn], FP32, tag=f"drel{cc}")
                    nc.sync.dma_start(out=dt_[:], in_=drel_in[rel, cc][:, :])
                    drel_s.append(dt_)

                calls = []
                for cc in range(NCHUNK):
                    tcn = int(T_c[cc])
                    calls.append(
                        [(t0, min(GT, tcn - t0)) for t0 in range(0, tcn, GT)]
                    )

                gbufs = [None] * NCHUNK
                gcall = [-1] * NCHUNK
                sbufs = [None] * NCHUNK
                sbatch = [-1] * NCHUNK

                def ensure_gather(cc, t):
                    k = t // GT
                    if gcall[cc] != k:
                        t0, nt = calls[cc][k]
                        ix = mp.tile([128, nt * 8], I16, tag=f"idx{cc}")
                        nc.sync.dma_start(
                            out=ix[:],
                            in_=idx_in[rel, cc][:, t0 * 8 : (t0 + nt) * 8],
                        )
                        gb = gpools[cc].tile([128, nt, 2 * D], BF16, tag=f"gb{cc}")
                        nc.gpsimd.dma_gather(
                            gb[:],
                            table[cc * CHUNK_ROWS : (cc + 1) * CHUNK_ROWS, :],
                            ix[:],
                            nt * 128,
                            nt * 128,
                            2 * D,
                            elem_step=2 * D,
                            queue_num=cc % NQUEUES,
                            single_packet=GT <= 8,
                        )
                        gbufs[cc] = gb
                        gcall[cc] = k
                    return gbufs[cc], t - calls[cc][k][0]

                def ensure_s(cc, t):
                    k = t // SB
                    if sbatch[cc] != k:
                        t0 = k * SB
                        nb = min(SB, int(T_c[cc]) - t0)
                        stile = sp.tile([128, SB, 128], BF16, tag=f"sb{cc}")
                        nc.vector.tensor_tensor(
                            out=stile[:, 0:nb, :],
                            in0=drel_s[cc][:, t0 : t0 + nb].to_broadcast(
                                [128, nb, 128]
                            ),
                            in1=iota_s[:]
                            .rearrange("p (o w) -> p o w", o=1)
                            .to_broadcast([128, nb, 128]),
                            op=mybir.AluOpType.is_equal,
                        )
                        sbufs[cc] = stile
                        sbatch[cc] = k
                    return sbufs[cc], t - k * SB

                if ABLATE == "gather":
                    for cc in range(NCHUNK):
                        for t in range(0, int(T_c[cc]), GT):
                            ensure_gather(cc, t)
                    return
                tile_cursor = [0] * NCHUNK
                for w in range(NW):
                    total_mms = int(ntiles_cw[:, w].sum())
                    ps = pw.tile([128, 128], FP32, tag="pw")
                    mm = 0
                    for cc in range(NCHUNK):
                        for _ in range(int(ntiles_cw[cc, w])):
                            t = tile_cursor[cc]
                            gb, gslot = ensure_gather(cc, t)
                            stile, sslot = ensure_s(cc, t)
                            nc.tensor.matmul(
                                ps[:],
                                gb[:, gslot, :],
                                stile[:, sslot, :],
                                start=(mm == 0),
                                stop=(mm == total_mms - 1),
                            )
                            tile_cursor[cc] += 1
                            mm += 1
                    nc.vector.tensor_copy(out=aggT[:, ts(w, 128)], in_=ps[0:D, :])

            def mlp_u():
                """xT = relu(wu2.T @ relu(wu1.T @ aggT + bu1) + bu2)."""
                for c0, cn in col_tiles:
                    ps1 = pu.tile([D, 512], FP32, tag="pu")
                    nc.tensor.matmul(
                        ps1[0:HU, :cn],
                        wu1_s[:],
                        aggT[:, c0 : c0 + cn],
                        start=True,
                        stop=True,
                    )
                    hu = sg.tile([HU, 512], FP32, tag="hu")
                    nc.scalar.activation(
                        hu[:, :cn], ps1[0:HU, :cn], AF.Relu, bias=bu1_s[:]
                    )
                    ps2 = pu.tile([D, 512], FP32, tag="pu")
                    nc.tensor.matmul(
                        ps2[:, :cn], wu2_s[:], hu[:, :cn], start=True, stop=True
                    )
                    nc.scalar.activation(
                        xT[0:D, c0 : c0 + cn], ps2[:, :cn], AF.Relu, bias=bu2_s[:]
                    )

            # ---------------- the 3 convs
            mlp_m()
            conv(meta_a, "a", tables[0])
            mlp_u()

            mlp_m()
            conv(meta_b, "b", tables[1])
            mlp_u()

            mlp_m()
            conv(meta_a, "a", tables[0])
            mlp_u()

            # ---------------- final h2o
            for j in range(NW):
                ps = pb.tile([128, D], FP32, tag="pb")
                nc.tensor.matmul(
                    ps[:], xT[:, ts(j, 128)], wob_s[:], start=True, stop=True
                )
                ostage = sg.tile([128, D], FP32, tag="ostage")
                nc.scalar.activation(ostage[:], ps[:], AF.Tanh)
                nc.sync.dma_start(out=out[ts(j, 128), :], in_=ostage[:])

    nc.compile()
    return nc


# ---------------------------------------------------------------- entry

def _prepare(
    x_served,
    x_interfered,
    edge_s2i,
    edge_i2s,
    wm1,
    bm1,
    wm2,
    bm2,
    wu1,
    bu1,
    wu2,
    bu2,
    wo,
    bo,
):
    """Host prep + program build. Returns (nc, in_maps)."""
    x_interfered = np.asarray(x_interfered, np.float32)
    e_s2i = np.asarray(edge_s2i)
    e_i2s = np.asarray(edge_i2s)

    # relation a: i2s (src interfered, dst served) -- convs 1 and 3
    meta_a = _prep_relation(e_i2s[0], e_i2s[1])
    # relation b: s2i (src served, dst interfered) -- conv 2
    meta_b = _prep_relation(e_s2i[0], e_s2i[1])

    nc = _build_program(meta_a, meta_b)

    wm2b = np.concatenate([wm2, bm2[None, :]], axis=0).astype(np.float32)
    wob = np.concatenate([wo, bo[None, :]], axis=0).astype(np.float32)

    in_maps = []
    for p in range(NCORES):
        xi_loc = np.zeros((D, PADPER), np.float32)
        xi_loc[:, :PERCORE] = x_interfered[p * PERCORE : (p + 1) * PERCORE].T
        m = {
            "xi0T": xi_loc,
            "wm1": np.ascontiguousarray(np.asarray(wm1, np.float32)),
            "bm1": np.ascontiguousarray(np.asarray(bm1, np.float32).reshape(HM, 1)),
            "wm2b": wm2b,
            "wu1": np.ascontiguousarray(np.asarray(wu1, np.float32)),
            "bu1": np.ascontiguousarray(np.asarray(bu1, np.float32).reshape(HU, 1)),
            "wu2": np.ascontiguousarray(np.asarray(wu2, np.float32)),
            "bu2": np.ascontiguousarray(np.asarray(bu2, np.float32).reshape(D, 1)),
            "wob": wob,
        }
        for rel, meta in (("a", meta_a), ("b", meta_b)):
            for cc in range(NCHUNK):
                m[f"idx_{rel}{cc}"] = meta["idx"][p][cc]
                m[f"drel_{rel}{cc}"] = meta["drel"][p][cc]
        in_maps.append(m)

    return nc, in_maps


def kernel(**inputs):
    from concourse.bass_utils import run_bass_kernel_spmd

    nc, in_maps = _prepare(**inputs)
    res = run_bass_kernel_spmd(
        nc, in_maps, core_ids=list(range(NCORES)), trace=TRACE
    )
    global LAST_RESULT
    LAST_RESULT = res
    outs = [res.results[p]["out"][:PERCORE] for p in range(NCORES)]
    return np.concatenate(outs, axis=0)


# revision 21
# speedup vs baseline: 2.0614x; 1.0166x over previous
"""FDGNN (gnn_message_passing) Trainium2 kernel, 8-core SPMD.

Strategy:
- Only 3 of the reference's 6 convs feed the output:
    s1 = conv_i2s(xi0); i2 = conv_s2i(s1); s3 = conv_i2s(i2); out = tanh(s3@wo+bo)
- mlp_m commutes with the per-edge gather: mlp_m(x[src]) == mlp_m(x)[src], so
  the message MLP runs per *node* (12.5K rows/core), not per *edge*.
- Destination nodes are sharded across the 8 cores. Each conv:
    1. Y_local = mlp_m(x_local)  (PE, feature-major in, row-major out)
    2. AllGather Y -> message table [100352, 64] in DRAM
    3. dma_gather edge source rows (4 int16-indexed table chunks)
    4. segment-sum: psum_aggT[64, w128] += gathered_tile.T @ S_tile, where
       S_tile is a 0/1 dst-selection matrix built on DVE via iota compare.
       Edges are pre-routed on host into per-(window, chunk) cells padded to
       128-slot tiles (pad slots gather zero rows).
    5. x_local' = mlp_u(aggT)  (feature-major all the way)
- Final: out = tanh(x @ wo + bo) per 128-row tile via the lhsT flip trick.
"""

import numpy as np

NCORES = 8
NNODE = 100000  # both NS and NI
PERCORE = NNODE // NCORES  # 12500
NW = 98  # windows per core (98*128 = 12544)
PADPER = NW * 128  # 12544 padded rows per core
TABLE_ROWS = NCORES * PADPER  # 100352
NCHUNK = 4
CHUNK_ROWS = TABLE_ROWS // NCHUNK  # 25088 (< 32768, int16-safe)
D = 64
HM = 32
HU = 16
import os as _os

GT = int(_os.environ.get("KGT", "24"))  # tiles per dma_gather call
NQUEUES = int(_os.environ.get("KNQ", "4"))  # SWDGE queues (1-4)
SCRATCH = int(_os.environ.get("KSCRATCH", "16384"))
SB = 8  # tiles per S-build batch

TRACE = False  # set by test harness to capture an NTFF profile
LAST_RESULT = None  # BassKernelResults of the most recent run

import os

ABLATE = os.environ.get("KABL", "")  # "", "mlponly", "ag", "gather"


# ---------------------------------------------------------------- host prep

def _prep_relation(src, dst):
    """Route edges (dst-sharded) into per-core, per-chunk gather streams."""
    E = src.shape[0]
    src = src.astype(np.int64)
    dst = dst.astype(np.int64)

    p = dst // PERCORE
    dl = dst - p * PERCORE
    trow = (src // PERCORE) * PADPER + (src % PERCORE)
    c = trow // CHUNK_ROWS
    lidx = trow - c * CHUNK_ROWS
    w = dl >> 7
    drel = dl - (w << 7)

    key = (p * NCHUNK + c) * NW + w
    counts = np.bincount(key, minlength=NCORES * NCHUNK * NW).reshape(
        NCORES, NCHUNK, NW
    )
    ntiles_cw = -(-counts.max(axis=0) // 128)  # [NCHUNK, NW]
    ntiles_cw[0] = np.maximum(ntiles_cw[0], 1)  # every window has >=1 tile
    N_cw = ntiles_cw * 128
    base_w = np.zeros((NCHUNK, NW + 1), np.int64)
    base_w[:, 1:] = np.cumsum(N_cw, axis=1)
    T_c = (base_w[:, -1] // 128).astype(np.int64)  # tiles per chunk stream

    # rank of each edge within its (p, c, w) cell
    order = np.argsort(key, kind="stable")
    kk = key[order]
    grp_first = np.r_[True, kk[1:] != kk[:-1]]
    first_pos = np.flatnonzero(grp_first)
    starts = np.repeat(first_pos, np.diff(np.r_[first_pos, E]))
    rank = np.arange(E) - starts
    inv = np.empty(E, np.int64)
    inv[order] = rank
    slot = base_w[c, w] + inv  # slot within (core, chunk) stream

    idx_streams = []  # [core][chunk] -> int16 [128, T_c*8] packed
    drel_streams = []  # [core][chunk] -> fp32 [128, T_c]
    for pp in range(NCORES):
        rows_i = []
        rows_d = []
        pm = p == pp
        for cc in range(NCHUNK):
            n = int(T_c[cc]) * 128
            ar = np.arange(n)
            idx_flat = (12500 + (ar % 44) + (ar % 2) * PADPER).astype(np.int64)
            idx_flat = np.minimum(idx_flat, CHUNK_ROWS - 1)
            drel_flat = np.full(n, -1.0, np.float32)
            m = pm & (c == cc)
            idx_flat[slot[m]] = lidx[m]
            drel_flat[slot[m]] = drel[m]
            assert idx_flat.max() < CHUNK_ROWS and idx_flat.min() >= 0
            idx16 = idx_flat.astype(np.int16)
            packed = np.tile(idx16.reshape(n // 16, 16).T, (8, 1))  # [128, n/16]
            rows_i.append(np.ascontiguousarray(packed))
            rows_d.append(
                np.ascontiguousarray(drel_flat.reshape(-1, 128).T.astype(np.float32))
            )
        idx_streams.append(rows_i)
        drel_streams.append(rows_d)

    return {
        "ntiles_cw": ntiles_cw,  # [NCHUNK, NW]
        "T_c": T_c,  # [NCHUNK]
        "idx": idx_streams,
        "drel": drel_streams,
    }


# ---------------------------------------------------------------- program

def _build_program(meta_a, meta_b):
    """meta_a: i2s relation (convs 1 and 3), meta_b: s2i relation (conv 2)."""
    import concourse.mybir as mybir
    import concourse.tile as tile
    from concourse import bacc
    from concourse.bass import ts

    FP32 = mybir.dt.float32
    BF16 = mybir.dt.bfloat16
    I16 = mybir.dt.int16
    AF = mybir.ActivationFunctionType

    nc = bacc.Bacc(
        "TRN2",
        target_bir_lowering=False,
        debug=False,
        enable_asserts=False,
        num_devices=NCORES,
        num_swdge_queues=NQUEUES,
        dynamic_dma_scratch_size=SCRATCH,
    )

    # ---- I/O
    xi0T = nc.dram_tensor("xi0T", [D, PADPER], FP32, kind="ExternalInput")
    wm1 = nc.dram_tensor("wm1", [D, HM], FP32, kind="ExternalInput")
    bm1 = nc.dram_tensor("bm1", [HM, 1], FP32, kind="ExternalInput")
    wm2b = nc.dram_tensor("wm2b", [HM + 1, D], FP32, kind="ExternalInput")
    wu1 = nc.dram_tensor("wu1", [D, HU], FP32, kind="ExternalInput")
    bu1 = nc.dram_tensor("bu1", [HU, 1], FP32, kind="ExternalInput")
    wu2 = nc.dram_tensor("wu2", [HU, D], FP32, kind="ExternalInput")
    bu2 = nc.dram_tensor("bu2", [D, 1], FP32, kind="ExternalInput")
    wob = nc.dram_tensor("wob", [D + 1, D], FP32, kind="ExternalInput")

    idx_in = {}
    drel_in = {}
    for rel, meta in (("a", meta_a), ("b", meta_b)):
        for cc in range(NCHUNK):
            tcn = int(meta["T_c"][cc])
            idx_in[rel, cc] = nc.dram_tensor(
                f"idx_{rel}{cc}", [128, tcn * 8], I16, kind="ExternalInput"
            )
            drel_in[rel, cc] = nc.dram_tensor(
                f"drel_{rel}{cc}", [128, tcn], FP32, kind="ExternalInput"
            )

    out = nc.dram_tensor("out", [PADPER, D], FP32, kind="ExternalOutput")

    # collective buffers; rows hold the 64 bf16 features twice (256B granule)
    y_bounce = nc.dram_tensor("y_bounce", [PADPER, 2 * D], BF16)
    tables = [
        nc.dram_tensor(f"table{i}", [TABLE_ROWS, 2 * D], BF16, addr_space="Shared")
        for i in range(2)
    ]

    iota_np = np.tile(np.arange(128, dtype=np.float32), (128, 1))
    iota_dram = nc.inline_tensor(iota_np, name="iota")
    import ml_dtypes

    ones_dram = nc.inline_tensor(np.ones((1, PADPER), np.float32), name="onesrow")
    zeros_dram = nc.inline_tensor(
        np.zeros((44, 2 * D), ml_dtypes.bfloat16), name="zerorows"
    )

    # PADPER = 12544 = 24*512 + 256
    col_tiles = [(i * 512, 512) for i in range(PADPER // 512)]
    if PADPER % 512:
        col_tiles.append((PADPER - PADPER % 512, PADPER % 512))

    with tile.TileContext(nc) as tc:
        with (
            tc.tile_pool(name="consts", bufs=1) as cs,
            tc.tile_pool(name="state", bufs=1) as st,
            tc.tile_pool(name="stage", bufs=3) as sg,
            tc.tile_pool(name="meta", bufs=2) as mp,
            tc.tile_pool(name="g0", bufs=2) as gp0,
            tc.tile_pool(name="g1", bufs=2) as gp1,
            tc.tile_pool(name="g2", bufs=2) as gp2,
            tc.tile_pool(name="g3", bufs=2) as gp3,
            tc.tile_pool(name="spool", bufs=2) as sp,
            tc.tile_pool(name="pw", bufs=2, space="PSUM") as pw,
            tc.tile_pool(name="pa", bufs=2, space="PSUM") as pa,
            tc.tile_pool(name="pb", bufs=2, space="PSUM") as pb,
            tc.tile_pool(name="pu", bufs=2, space="PSUM") as pu,
        ):
            gpools = [gp0, gp1, gp2, gp3]

            # ---- constants
            iota_s = cs.tile([128, 128], FP32)
            nc.sync.dma_start(out=iota_s[:], in_=iota_dram[:, :])
            wm1_s = cs.tile([D, HM], FP32)
            nc.sync.dma_start(out=wm1_s[:], in_=wm1[:, :])
            bm1_s = cs.tile([HM, 1], FP32)
            nc.sync.dma_start(out=bm1_s[:], in_=bm1[:, :])
            wm2b_s = cs.tile([HM + 1, D], FP32)
            nc.sync.dma_start(out=wm2b_s[:], in_=wm2b[:, :])
            wu1_s = cs.tile([D, HU], FP32)
            nc.sync.dma_start(out=wu1_s[:], in_=wu1[:, :])
            bu1_s = cs.tile([HU, 1], FP32)
            nc.sync.dma_start(out=bu1_s[:], in_=bu1[:, :])
            wu2_s = cs.tile([HU, D], FP32)
            nc.sync.dma_start(out=wu2_s[:], in_=wu2[:, :])
            bu2_s = cs.tile([D, 1], FP32)
            nc.sync.dma_start(out=bu2_s[:], in_=bu2[:, :])
            wob_s = cs.tile([D + 1, D], FP32)
            nc.sync.dma_start(out=wob_s[:], in_=wob[:, :])

            # ---- persistent state
            xT = st.tile([D + 1, PADPER], FP32)  # row D = ones
            nc.sync.dma_start(out=xT[0:D, :], in_=xi0T[:, :])
            nc.sync.dma_start(out=xT[D : D + 1, :], in_=ones_dram[:, :])
            aggT = st.tile([D, PADPER], FP32)
            if ABLATE:
                nc.gpsimd.memset(aggT[:], 0.0)

            # zero rows of the y bounce buffer (pad rows 12500..12543), once
            zrow = cs.tile([44, 2 * D], BF16)
            nc.sync.dma_start(out=zrow[:], in_=zeros_dram[:, :])
            nc.sync.dma_start(out=y_bounce[12500:PADPER, :], in_=zrow[:])

            def mlp_m():
                """y_bounce = mlp_m(x) row-major via flipped second matmul."""
                h1_tiles = {}
                for c0, cn in col_tiles:
                    ps = pa.tile([HM, 512], FP32, tag="pa")
                    nc.tensor.matmul(
                        ps[:, :cn],
                        wm1_s[:],
                        xT[0:D, c0 : c0 + cn],
                        start=True,
                        stop=True,
                    )
                    h1 = sg.tile([HM + 1, 512], FP32, tag="h1")
                    nc.scalar.activation(
                        h1[0:HM, :cn], ps[:, :cn], AF.Relu, bias=bm1_s[:]
                    )
                    nc.sync.dma_start(
                        out=h1[HM : HM + 1, :cn], in_=ones_dram[:, 0:cn]
                    )
                    h1_tiles[c0] = h1
                    for j0 in range(0, cn, 128):
                        j = (c0 + j0) // 128
                        ps2 = pb.tile([128, D], FP32, tag="pb")
                        nc.tensor.matmul(
                            ps2[:],
                            h1[:, j0 : j0 + 128],
                            wm2b_s[:],
                            start=True,
                            stop=True,
                        )
                        ystage = sg.tile([128, D], BF16, tag="ystage")
                        nc.scalar.activation(ystage[:], ps2[:], AF.Relu)
                        r0 = j * 128
                        nrows = 128 if j < NW - 1 else (12500 - r0)
                        nc.sync.dma_start(
                            out=y_bounce[r0 : r0 + nrows, 0:D], in_=ystage[0:nrows, :]
                        )
                        nc.sync.dma_start(
                            out=y_bounce[r0 : r0 + nrows, D : 2 * D],
                            in_=ystage[0:nrows, :],
                        )

            def conv(meta, rel, table):
                """AllGather y -> table; gather + segment-sum -> aggT."""
                if ABLATE == "mlponly":
                    return
                nc.gpsimd.collective_compute(
                    "AllGather",
                    mybir.AluOpType.bypass,
                    replica_groups=[list(range(NCORES))],
                    ins=[y_bounce.ap().opt()],
                    outs=[table.ap().opt()],
                )

                if ABLATE == "ag":
                    return
                ntiles_cw = meta["ntiles_cw"]
                T_c = meta["T_c"]

                # drel streams loaded whole (small)
                drel_s = []
                for cc in range(NCHUNK):
                    tcn = int(T_c[cc])
                    dt_ = mp.tile([128, tcn], FP32, tag=f"drel{cc}")
                    nc.sync.dma_start(out=dt_[:], in_=drel_in[rel, cc][:, :])
                    drel_s.append(dt_)

                calls = []
                for cc in range(NCHUNK):
                    tcn = int(T_c[cc])
                    calls.append(
                        [(t0, min(GT, tcn - t0)) for t0 in range(0, tcn, GT)]
                    )

                gbufs = [None] * NCHUNK
                gcall = [-1] * NCHUNK
                sbufs = [None] * NCHUNK
                sbatch = [-1] * NCHUNK

                def ensure_gather(cc, t):
                    k = t // GT
                    if gcall[cc] != k:
                        t0, nt = calls[cc][k]
                        ix = mp.tile([128, nt * 8], I16, tag=f"idx{cc}")
                        nc.sync.dma_start(
                            out=ix[:],
                            in_=idx_in[rel, cc][:, t0 * 8 : (t0 + nt) * 8],
                        )
                        gb = gpools[cc].tile([128, nt, 2 * D], BF16, tag=f"gb{cc}")
                        nc.gpsimd.dma_gather(
                            gb[:],
                            table[cc * CHUNK_ROWS : (cc + 1) * CHUNK_ROWS, :],
                            ix[:],
                            nt * 128,
                            nt * 128,
                            2 * D,
                            elem_step=2 * D,
                            queue_num=cc % NQUEUES,
                            single_packet=GT <= 8,
                        )
                        gbufs[cc] = gb
                        gcall[cc] = k
                    return gbufs[cc], t - calls[cc][k][0]

                def ensure_s(cc, t):
                    k = t // SB
                    if sbatch[cc] != k:
                        t0 = k * SB
                        nb = min(SB, int(T_c[cc]) - t0)
                        stile = sp.tile([128, SB, 128], BF16, tag=f"sb{cc}")
                        nc.vector.tensor_tensor(
                            out=stile[:, 0:nb, :],
                            in0=drel_s[cc][:, t0 : t0 + nb].to_broadcast(
                                [128, nb, 128]
                            ),
                            in1=iota_s[:]
                            .rearrange("p (o w) -> p o w", o=1)
                            .to_broadcast([128, nb, 128]),
                            op=mybir.AluOpType.is_equal,
                        )
                        sbufs[cc] = stile
                        sbatch[cc] = k
                    return sbufs[cc], t - k * SB

                if ABLATE == "gather":
                    for cc in range(NCHUNK):
                        for t in range(0, int(T_c[cc]), GT):
                            ensure_gather(cc, t)
                    return
                tile_cursor = [0] * NCHUNK
                for w in range(NW):
                    total_mms = int(ntiles_cw[:, w].sum())
                    ps = pw.tile([128, 128], FP32, tag="pw")
                    mm = 0
                    for cc in range(NCHUNK):
                        for _ in range(int(ntiles_cw[cc, w])):
                            t = tile_cursor[cc]
                            gb, gslot = ensure_gather(cc, t)
                            stile, sslot = ensure_s(cc, t)
                            nc.tensor.matmul(
                                ps[:],
                                gb[:, gslot, :],
                                stile[:, sslot, :],
                                start=(mm == 0),
                                stop=(mm == total_mms - 1),
                            )
                            tile_cursor[cc] += 1
                            mm += 1
                    nc.vector.tensor_copy(out=aggT[:, ts(w, 128)], in_=ps[0:D, :])

            def mlp_u():
                """xT = relu(wu2.T @ relu(wu1.T @ aggT + bu1) + bu2)."""
                for c0, cn in col_tiles:
                    ps1 = pu.tile([D, 512], FP32, tag="pu")
                    nc.tensor.matmul(
                        ps1[0:HU, :cn],
                        wu1_s[:],
                        aggT[:, c0 : c0 + cn],
                        start=True,
                        stop=True,
                    )
                    hu = sg.tile([HU, 512], FP32, tag="hu")
                    nc.scalar.activation(
                        hu[:, :cn], ps1[0:HU, :cn], AF.Relu, bias=bu1_s[:]
                    )
                    ps2 = pu.tile([D, 512], FP32, tag="pu")
                    nc.tensor.matmul(
                        ps2[:, :cn], wu2_s[:], hu[:, :cn], start=True, stop=True
                    )
                    nc.scalar.activation(
                        xT[0:D, c0 : c0 + cn], ps2[:, :cn], AF.Relu, bias=bu2_s[:]
                    )

            # ---------------- the 3 convs
            mlp_m()
            conv(meta_a, "a", tables[0])
            mlp_u()

            mlp_m()
            conv(meta_b, "b", tables[1])
            mlp_u()

            mlp_m()
            conv(meta_a, "a", tables[0])
            mlp_u()

            # ---------------- final h2o
            for j in range(NW):
                ps = pb.tile([128, D], FP32, tag="pb")
                nc.tensor.matmul(
                    ps[:], xT[:, ts(j, 128)], wob_s[:], start=True, stop=True
                )
                ostage = sg.tile([128, D], FP32, tag="ostage")
                nc.scalar.activation(ostage[:], ps[:], AF.Tanh)
                nc.sync.dma_start(out=out[ts(j, 128), :], in_=ostage[:])

    nc.compile()
    return nc


# ---------------------------------------------------------------- entry

def _prepare(
    x_served,
    x_interfered,
    edge_s2i,
    edge_i2s,
    wm1,
    bm1,
    wm2,
    bm2,
    wu1,
    bu1,
    wu2,
    bu2,
    wo,
    bo,
):
    """Host prep + program build. Returns (nc, in_maps)."""
    x_interfered = np.asarray(x_interfered, np.float32)
    e_s2i = np.asarray(edge_s2i)
    e_i2s = np.asarray(edge_i2s)

    # relation a: i2s (src interfered, dst served) -- convs 1 and 3
    meta_a = _prep_relation(e_i2s[0], e_i2s[1])
    # relation b: s2i (src served, dst interfered) -- conv 2
    meta_b = _prep_relation(e_s2i[0], e_s2i[1])

    nc = _build_program(meta_a, meta_b)

    wm2b = np.concatenate([wm2, bm2[None, :]], axis=0).astype(np.float32)
    wob = np.concatenate([wo, bo[None, :]], axis=0).astype(np.float32)

    in_maps = []
    for p in range(NCORES):
        xi_loc = np.zeros((D, PADPER), np.float32)
        xi_loc[:, :PERCORE] = x_interfered[p * PERCORE : (p + 1) * PERCORE].T
        m = {
            "xi0T": xi_loc,
            "wm1": np.ascontiguousarray(np.asarray(wm1, np.float32)),
            "bm1": np.ascontiguousarray(np.asarray(bm1, np.float32).reshape(HM, 1)),
            "wm2b": wm2b,
            "wu1": np.ascontiguousarray(np.asarray(wu1, np.float32)),
            "bu1": np.ascontiguousarray(np.asarray(bu1, np.float32).reshape(HU, 1)),
            "wu2": np.ascontiguousarray(np.asarray(wu2, np.float32)),
            "bu2": np.ascontiguousarray(np.asarray(bu2, np.float32).reshape(D, 1)),
            "wob": wob,
        }
        for rel, meta in (("a", meta_a), ("b", meta_b)):
            for cc in range(NCHUNK):
                m[f"idx_{rel}{cc}"] = meta["idx"][p][cc]
                m[f"drel_{rel}{cc}"] = meta["drel"][p][cc]
        in_maps.append(m)

    return nc, in_maps


def kernel(**inputs):
    from concourse.bass_utils import run_bass_kernel_spmd

    nc, in_maps = _prepare(**inputs)
    res = run_bass_kernel_spmd(
        nc, in_maps, core_ids=list(range(NCORES)), trace=TRACE
    )
    global LAST_RESULT
    LAST_RESULT = res
    outs = [res.results[p]["out"][:PERCORE] for p in range(NCORES)]
    return np.concatenate(outs, axis=0)
